# revision 1
# baseline (speedup 1.0000x reference)
"""Trainium2 Bass kernel for nn_EventPairCompositionModel.

Strategy (data-parallel over batch, 8 cores, B=512 -> 64 per core):
  - Host compacts the 60MB f32 table per core to the ~24K unique rows its
    shard touches (bf16, rows padded to 384 elems = 768B), remapping all
    indices to int16.  The device then uses the fast SWDGE dma_gather
    (InstDMAGatherAnt) to fetch context/event embeddings.
  - XBAR DMA transpose (SBUF->SBUF) turns gathered bn-major rows into
    K-major tiles for the tensor engine.
  - Shared arg-composition MLP (1536->512->256, zero-padded K) in bf16.
  - Cosine numerators/denominators via small per-b matmuls that land
    n-on-partitions; norms folded together through one exp(-0.5 ln x).
  - KNRM kernel pooling via ones-matmul partition reductions, distance
    kernel path, final linear + sigmoid, all on-chip.
  - If a shard ever touches >32767 unique rows (can't happen for random
    inputs), falls back to a slow indirect-DMA gather of the full table.
All 8 cores run the identical program on their own batch shard (SPMD, no
collectives); host concatenates the 8 (64,1) outputs.
"""

import numpy as np
import ml_dtypes

import concourse.bacc as bacc
import concourse.bass as bass
import concourse.tile as tile
import concourse.mybir as mybir
from concourse.bass import IndirectOffsetOnAxis
from concourse.bass_utils import run_bass_kernel_spmd
from concourse import library_config

F32 = mybir.dt.float32
BF16 = mybir.dt.bfloat16
I16 = mybir.dt.int16
I32 = mybir.dt.int32
AF = mybir.ActivationFunctionType

# Problem shapes (hardcoded per spec)
B, N, C, E = 512, 128, 4, 300
V = 50000
H1, H2 = 512, 256
NF, NK = 8, 11
NCORES = 8
BC = B // NCORES          # 64 batches per core
EP = 384                  # padded embedding stride inside an x-row (768B)
CE = C * EP               # 1536 padded x-row length
KT = CE // 128            # 12 K-tiles for MLP1
CT = 32768                # compact table rows (int16-indexable)
GROUPS = (BC * N) // 512  # 16 groups of 512 (b,n) pairs
SUBT = 4                  # 128-bn subtiles per group
EB = 128                  # event-path width (64 real b + 64 junk)

MUS = [1.0, 0.9, 0.7, 0.5, 0.3, 0.1, -0.1, -0.3, -0.5, -0.7, -0.9]
SIGMAS = [1e-3] + [0.1] * 10

_PROGRAM_CACHE = {}


def _build_program(fast: bool):
    if fast in _PROGRAM_CACHE:
        return _PROGRAM_CACHE[fast]

    nc = bacc.Bacc("TRN2", target_bir_lowering=False, debug=False, num_swdge_queues=4)

    # ---- DRAM I/O ----
    if fast:
        ctab = nc.dram_tensor("ctab", (CT, EP), BF16, kind="ExternalInput")
        cidx = nc.dram_tensor("cidx", (128, GROUPS * 128), I16, kind="ExternalInput")
        eidx = nc.dram_tensor("eidx", (128, 32), I16, kind="ExternalInput")
    else:
        ctab = nc.dram_tensor("table", (V + 1, E), F32, kind="ExternalInput")
        cidx = nc.dram_tensor("ctxidx", (128, BC * C), I32, kind="ExternalInput")
        eidx = nc.dram_tensor("evidx", (BC, C), I32, kind="ExternalInput")
    w1t = nc.dram_tensor("w1t", (CE, H1), BF16, kind="ExternalInput")
    w2t = nc.dram_tensor("w2t", (H1, H2), BF16, kind="ExternalInput")
    wvt = nc.dram_tensor("wvt", (CE, 9), BF16, kind="ExternalInput")
    b1d = nc.dram_tensor("b1d", (128, 4), F32, kind="ExternalInput")
    b2d = nc.dram_tensor("b2d", (128, 2), F32, kind="ExternalInput")
    bvd = nc.dram_tensor("bvd", (9, 1), F32, kind="ExternalInput")
    wct = nc.dram_tensor("wct", (128, 1), F32, kind="ExternalInput")
    wckp = nc.dram_tensor("wckp", (1, NK), F32, kind="ExternalInput")
    bcd = nc.dram_tensor("bcd", (1, 1), F32, kind="ExternalInput")
    ndsq = nc.dram_tensor("ndsq", (9, BC), F32, kind="ExternalInput")
    featT = nc.dram_tensor("featT", (NF, BC), F32, kind="ExternalInput")
    out_d = nc.dram_tensor("out", (BC, 1), F32, kind="ExternalOutput")

    with tile.TileContext(nc) as tc:
        with (
            tc.tile_pool(name="consts", bufs=1) as cpool,
            tc.tile_pool(name="xg", bufs=4) as xgpool,
            tc.tile_pool(name="xt", bufs=4) as xtpool,
            tc.tile_pool(name="s1", bufs=8) as s1pool,
            tc.tile_pool(name="s2", bufs=4) as s2pool,
            tc.tile_pool(name="csq", bufs=4) as csqpool,
            tc.tile_pool(name="small", bufs=2) as smpool,
            tc.tile_pool(name="pm1", bufs=2, space="PSUM") as pm1,
            tc.tile_pool(name="pm2", bufs=2, space="PSUM") as pm2,
            tc.tile_pool(name="ptn", bufs=1, space="PSUM") as ptn,
            tc.tile_pool(name="pmisc", bufs=2, space="PSUM") as pmisc,
        ):
            # ---- load constants ----
            if fast:
                nc.gpsimd.load_library(library_config.mlp)
                cidx_s = cpool.tile([128, GROUPS * 128], I16)
                nc.sync.dma_start(cidx_s[:], cidx.ap())
                eidx_s = cpool.tile([128, 32], I16)
                nc.sync.dma_start(eidx_s[:], eidx.ap())
            w1t_s = cpool.tile([128, KT * H1], BF16)
            nc.sync.dma_start(
                w1t_s[:].rearrange("p (t m) -> p t m", t=KT),
                w1t.ap().rearrange("(t p) m -> p t m", p=128),
            )
            w2t_s = cpool.tile([128, 4 * H2], BF16)
            nc.scalar.dma_start(
                w2t_s[:].rearrange("p (t m) -> p t m", t=4),
                w2t.ap().rearrange("(t p) m -> p t m", p=128),
            )
            wvt_s = cpool.tile([128, KT * 9], BF16)
            nc.scalar.dma_start(
                wvt_s[:].rearrange("p (t m) -> p t m", t=KT),
                wvt.ap().rearrange("(t p) m -> p t m", p=128),
            )
            b1_s = cpool.tile([128, 4], F32)
            nc.sync.dma_start(b1_s[:], b1d.ap())
            b2_s = cpool.tile([128, 2], F32)
            nc.sync.dma_start(b2_s[:], b2d.ap())
            bv_s = cpool.tile([9, 1], F32)
            nc.sync.dma_start(bv_s[:], bvd.ap())
            wct_s = cpool.tile([128, 1], F32)
            nc.sync.dma_start(wct_s[:], wct.ap())
            wckp_s = cpool.tile([1, NK], F32)
            nc.sync.dma_start(wckp_s[:], wckp.ap())
            bc_s = cpool.tile([1, 1], F32)
            nc.sync.dma_start(bc_s[:], bcd.ap())
            if not fast:
                cidx_s = cpool.tile([128, BC * C], I32)
                nc.sync.dma_start(cidx_s[:], cidx.ap())
                eidx_s = cpool.tile([BC, C], I32)
                nc.sync.dma_start(eidx_s[:], eidx.ap())
            ndsq_s = cpool.tile([9, BC], F32)
            nc.sync.dma_start(ndsq_s[:], ndsq.ap())
            feat_s = cpool.tile([128, BC], F32)
            nc.vector.memset(feat_s[:], 0.0)
            nc.sync.dma_start(feat_s[64 : 64 + NF, :], featT.ap())
            ones_s = cpool.tile([128, 1], BF16)
            nc.vector.memset(ones_s[:], 1.0)
            onesrow_s = cpool.tile([1, 128], F32)
            nc.vector.memset(onesrow_s[:], 1.0)
            onesf_s = cpool.tile([128, 1], F32)
            nc.vector.memset(onesf_s[:], 1.0)
            eps_s = cpool.tile([128, 1], F32)
            nc.vector.memset(eps_s[:], 1e-20)
            mub_s = cpool.tile([128, NK], F32)
            for k in range(NK):
                nc.vector.memset(mub_s[:, k : k + 1], -MUS[k])

            # ---- event path (EB=128 lanes, only 0..63 meaningful) ----
            xeT = cpool.tile([128, KT * EB], BF16)
            if fast:
                # transpose-mode gather lands K-major directly:
                # xeT[p, jj, c*128+b] = emb_{b,c}[jj*128+p]
                nc.gpsimd.dma_gather(
                    out_ap=xeT[:].rearrange("p (j i) -> p j i", j=3),
                    in_ap=ctab.ap(),
                    idxs_ap=eidx_s[:],
                    num_idxs=512,
                    num_idxs_reg=512,
                    elem_size=EP,
                    transpose=True,
                )
            else:
                xe = cpool.tile([EB, CE], BF16)
                nc.vector.memset(xe[:], 0.0)
                nc.gpsimd.indirect_dma_start(
                    out=xe[0:BC, :].rearrange("p (c e) -> p c e", c=C)[:, :, 0:E],
                    out_offset=None,
                    in_=ctab.ap(),
                    in_offset=IndirectOffsetOnAxis(ap=eidx_s[:], axis=0),
                )
                nc.sync.dma_start_transpose(
                    xeT[:].rearrange("p (j i) -> p j i", j=KT), xe[:]
                )

            def xeT_k(j):
                # K-tile j = 3*c + jj of the event activations
                if fast:
                    return xeT[:, 512 * (j % 3) + 128 * (j // 3) :][:, 0:128]
                return xeT[:, EB * j : EB * (j + 1)]

            s1e = cpool.tile([128, 4 * EB], BF16)
            for m in range(4):
                pe = pmisc.tile([128, EB], F32, tag="pmisc", name="pe")
                for j in range(KT):
                    nc.tensor.matmul(
                        pe[:],
                        w1t_s[:, H1 * j + 128 * m : H1 * j + 128 * m + 128],
                        xeT_k(j),
                        start=(j == 0),
                        stop=(j == KT - 1),
                    )
                nc.scalar.activation(
                    s1e[:, EB * m : EB * (m + 1)], pe[:], AF.Relu,
                    bias=b1_s[:, m : m + 1],
                )

            eh2 = [
                cpool.tile([128, EB], BF16, tag=f"eh2_{k}", name=f"eh2_{k}")
                for k in range(2)
            ]
            for m in range(2):
                pe2 = pmisc.tile([128, EB], F32, tag="pmisc", name="pe2")
                for j in range(4):
                    nc.tensor.matmul(
                        pe2[:],
                        w2t_s[:, H2 * j + 128 * m : H2 * j + 128 * m + 128],
                        s1e[:, EB * j : EB * (j + 1)],
                        start=(j == 0),
                        stop=(j == 3),
                    )
                nc.scalar.activation(
                    eh2[m][:], pe2[:], AF.Relu, bias=b2_s[:, m : m + 1]
                )

            # variances -> dist_emb rows 32..40 of feat_s
            pv = pmisc.tile([9, EB], F32, tag="pmisc", name="pv")
            for j in range(KT):
                nc.tensor.matmul(
                    pv[:],
                    wvt_s[:, 9 * j : 9 * (j + 1)],
                    xeT_k(j),
                    start=(j == 0),
                    stop=(j == KT - 1),
                )
            ez_s = smpool.tile([9, EB], F32)
            nc.scalar.activation(ez_s[:], pv[:], AF.Exp, bias=bv_s[:])
            ez1_s = smpool.tile([9, EB], F32)
            nc.vector.tensor_scalar_add(ez1_s[:], ez_s[:], 1.0)
            var_s = smpool.tile([9, EB], F32)
            nc.scalar.activation(var_s[:], ez1_s[:], AF.Ln)
            rv_s = smpool.tile([9, EB], F32)
            nc.vector.reciprocal(rv_s[:], var_s[:])
            q_s = smpool.tile([9, BC], F32)
            nc.vector.tensor_mul(q_s[:], ndsq_s[:], rv_s[:, 0:BC])
            nc.scalar.activation(feat_s[32:41, :], q_s[:], AF.Exp)

            # |e|^2 per b, broadcast to all 128 partitions via outer product
            esq = [
                smpool.tile([128, EB], BF16, tag=f"esq_{k}", name=f"esq_{k}")
                for k in range(2)
            ]
            for k in range(2):
                nc.vector.tensor_mul(esq[k][:], eh2[k][:], eh2[k][:])
            pne = pmisc.tile([1, EB], F32, tag="pmisc", name="pne")
            for k in range(2):
                nc.tensor.matmul(
                    pne[:], ones_s[:], esq[k][:], start=(k == 0), stop=(k == 1)
                )
            ne2_s = smpool.tile([1, BC], F32)
            nc.scalar.copy(ne2_s[:], pne[:, 0:BC])
            pne2bc = pmisc.tile([128, BC], F32, tag="pmisc", name="pne2bc")
            nc.tensor.matmul(
                pne2bc[:], onesrow_s[:], ne2_s[:], start=True, stop=True
            )
            ne2bc_s = cpool.tile([128, BC], F32)
            nc.scalar.copy(ne2bc_s[:], pne2bc[:])

            # persistent SBUF accumulators, n on partitions, b on free
            traw_s = cpool.tile([128, BC], F32)
            ncsq_s = cpool.tile([128, BC], F32)

            # ---- context groups ----
            for g in range(GROUPS):
                xt = xtpool.tile([128, KT * 512], BF16)
                if fast:
                    # per subtile s: xt[p, s, jj, c*128+pbn] (s-major blocks)
                    for s in range(SUBT):
                        nc.gpsimd.dma_gather(
                            out_ap=xt[:]
                            .rearrange("p (z j i) -> p z j i", z=SUBT, j=3)[
                                :, s, :, :
                            ],
                            in_ap=ctab.ap(),
                            idxs_ap=cidx_s[
                                :, 32 * (SUBT * g + s) : 32 * (SUBT * g + s + 1)
                            ],
                            num_idxs=512,
                            num_idxs_reg=512,
                            elem_size=EP,
                            transpose=True,
                        )
                else:
                    xg = xgpool.tile([128, SUBT * CE], BF16)
                    nc.vector.memset(
                        xg[:].rearrange("p (q e) -> p q e", e=EP)[:, :, E:EP],
                        0.0,
                    )
                    for s in range(SUBT):
                        nc.gpsimd.indirect_dma_start(
                            out=xg[:]
                            .rearrange("p (q c e) -> p q c e", q=SUBT, c=C)[
                                :, s, :, 0:E
                            ],
                            out_offset=None,
                            in_=ctab.ap(),
                            in_offset=IndirectOffsetOnAxis(
                                ap=cidx_s[
                                    :, (SUBT * g + s) * C : (SUBT * g + s + 1) * C
                                ],
                                axis=0,
                            ),
                        )
                    for s in range(SUBT):
                        nc.sync.dma_start_transpose(
                            xt[:].rearrange(
                                "p (j z i) -> p j z i", j=KT, z=SUBT
                            )[:, :, s, :],
                            xg[:, CE * s : CE * (s + 1)],
                        )

                def xt_k(j):
                    # K-tile j = 3*c + jj; cols ordered (s, pbn)
                    if fast:
                        off = 512 * (j % 3) + 128 * (j // 3)
                        return xt[:].rearrange(
                            "p (z x) -> p z x", z=SUBT
                        )[:, :, off : off + 128]
                    return xt[:, 512 * j : 512 * (j + 1)]

                s1 = [
                    s1pool.tile([128, 512], BF16, tag=f"s1_{m}", name=f"s1_{m}")
                    for m in range(4)
                ]
                for m in range(4):
                    p1 = pm1.tile([128, 512], F32)
                    for j in range(KT):
                        nc.tensor.matmul(
                            p1[:],
                            w1t_s[:, H1 * j + 128 * m : H1 * j + 128 * m + 128],
                            xt_k(j),
                            start=(j == 0),
                            stop=(j == KT - 1),
                        )
                    nc.scalar.activation(
                        s1[m][:], p1[:], AF.Relu, bias=b1_s[:, m : m + 1]
                    )

                s2 = [
                    s2pool.tile([128, 512], BF16, tag=f"s2_{m}", name=f"s2_{m}")
                    for m in range(2)
                ]
                for m in range(2):
                    p2 = pm2.tile([128, 512], F32)
                    for j in range(4):
                        nc.tensor.matmul(
                            p2[:],
                            w2t_s[:, H2 * j + 128 * m : H2 * j + 128 * m + 128],
                            s1[j][:],
                            start=(j == 0),
                            stop=(j == 3),
                        )
                    nc.scalar.activation(
                        s2[m][:], p2[:], AF.Relu, bias=b2_s[:, m : m + 1]
                    )

                csq = [
                    csqpool.tile([128, 512], BF16, tag=f"csq_{m}", name=f"csq_{m}")
                    for m in range(2)
                ]
                for m in range(2):
                    nc.vector.tensor_mul(csq[m][:], s2[m][:], s2[m][:])

                # raw dots and |c|^2, n on partitions, one column per b
                pT = ptn.tile([128, SUBT], F32, tag="pT", name="pT")
                pN = ptn.tile([128, SUBT], F32, tag="pN", name="pN")
                for s in range(SUBT):
                    b = SUBT * g + s
                    for k in range(2):
                        nc.tensor.matmul(
                            pT[:, s : s + 1],
                            s2[k][:, 128 * s : 128 * (s + 1)],
                            eh2[k][:, b : b + 1],
                            start=(k == 0),
                            stop=(k == 1),
                        )
                    for k in range(2):
                        nc.tensor.matmul(
                            pN[:, s : s + 1],
                            csq[k][:, 128 * s : 128 * (s + 1)],
                            ones_s[:],
                            start=(k == 0),
                            stop=(k == 1),
                        )
                nc.scalar.copy(traw_s[:, SUBT * g : SUBT * (g + 1)], pT[:])
                nc.scalar.copy(ncsq_s[:, SUBT * g : SUBT * (g + 1)], pN[:])

            # ---- kernel pooling (tiles are [n=128, b=64]) ----
            prodn = smpool.tile([128, BC], F32, tag="prodn")
            nc.vector.tensor_mul(prodn[:], ncsq_s[:], ne2bc_s[:])
            lnp = smpool.tile([128, BC], F32, tag="lnp")
            nc.scalar.activation(lnp[:], prodn[:], AF.Ln, bias=eps_s[:])
            nrmf = smpool.tile([128, BC], F32, tag="nrmf")
            nc.scalar.activation(nrmf[:], lnp[:], AF.Exp, scale=-0.5)
            trans = cpool.tile([128, BC], F32)
            nc.vector.tensor_mul(trans[:], traw_s[:], nrmf[:])

            kpp_s = cpool.tile([1, NK * BC], F32)
            for k in range(NK):
                sq = smpool.tile([128, BC], F32, tag="sq", name="sq")
                nc.scalar.activation(
                    sq[:], trans[:], AF.Square, bias=mub_s[:, k : k + 1]
                )
                arg = smpool.tile([128, BC], F32, tag="arg", name="arg")
                nc.vector.tensor_scalar(
                    arg[:], sq[:],
                    -1.0 / (2.0 * SIGMAS[k] ** 2), -87.0,
                    mybir.AluOpType.mult, mybir.AluOpType.max,
                )
                ek = smpool.tile([128, BC], F32, tag="ek", name="ek")
                nc.scalar.activation(ek[:], arg[:], AF.Exp)
                pp = pmisc.tile([1, BC], F32, tag="pmisc", name="pp")
                nc.tensor.matmul(pp[:], onesf_s[:], ek[:], start=True, stop=True)
                nc.scalar.copy(kpp_s[:, BC * k : BC * (k + 1)], pp[:])

            kpc_s = smpool.tile([1, NK * BC], F32, tag="kpc")
            nc.vector.tensor_scalar_max(kpc_s[:], kpp_s[:], 1e-10)
            kpl_s = smpool.tile([1, NK * BC], F32, tag="kpl")
            nc.scalar.activation(kpl_s[:], kpc_s[:], AF.Ln)

            # weighted sum over k: kps[b] = sum_k wckp[k] * kpl[k, b]
            kpw_s = smpool.tile([1, BC * NK], F32, tag="kpw")
            kpl_v = kpl_s[:].rearrange("p (k b) -> p b k", k=NK)
            wck_v = wckp_s[:][:, None, :].broadcast_to([1, BC, NK])
            kpw_v = kpw_s[:].rearrange("p (b k) -> p b k", b=BC)
            nc.vector.tensor_tensor(
                out=kpw_v, in0=kpl_v, in1=wck_v, op=mybir.AluOpType.mult
            )
            kps_s = smpool.tile([1, BC], F32, tag="kps")
            nc.vector.reduce_sum(
                out=kps_s[:], in_=kpw_v, axis=mybir.AxisListType.X
            )

            # ---- final score ----
            psc = pmisc.tile([1, BC], F32, tag="pmisc", name="psc")
            nc.tensor.matmul(psc[:], wct_s[:], feat_s[:], start=True, stop=True)
            tot_s = smpool.tile([1, BC], F32, tag="tot")
            nc.vector.tensor_add(tot_s[:], psc[:], kps_s[:])
            emx = smpool.tile([1, BC], F32, tag="emx")
            nc.scalar.activation(emx[:], tot_s[:], AF.Exp, bias=bc_s[:], scale=-1.0)
            emx1 = smpool.tile([1, BC], F32, tag="emx1")
            nc.vector.tensor_scalar_add(emx1[:], emx[:], 1.0)
            outs = smpool.tile([1, BC], F32, tag="outs")
            nc.vector.reciprocal(outs[:], emx1[:])
            nc.sync.dma_start(out_d.ap().rearrange("b one -> one b"), outs[:])

    nc.compile()

    # Spread SWDGE gathers across the 4 queues. The ucode locks each DMASW
    # semaphore lane to one queue, and Tile assigns lanes round-robin in
    # scheduled order, so derive queue from the assigned lane post-compile.
    import re as _re
    for blk in nc.m.functions[0].blocks:
        for inst in blk.instructions:
            if type(inst).__name__ == "InstDMAGatherAnt":
                for u in inst.sync_info.on_update:
                    m = _re.match(r"DMASW(\d+)_", u.ant_name or "")
                    if m:
                        inst.queue_num = int(m.group(1)) % 4
                        break

    _PROGRAM_CACHE[fast] = nc
    return nc


def _wrap16(flat_idx):
    """int16 index list -> (128, n/16) tile layout: unwrapped[i] =
    tile[i % 16, i // 16], replicated into all 8 16-partition stripes."""
    n = flat_idx.shape[0]
    t = np.zeros((16, n // 16), np.int16)
    t[np.arange(n) % 16, np.arange(n) // 16] = flat_idx
    return np.tile(t, (8, 1))


def _prep_core_inputs(inputs, core, fast):
    """Host-side shard + weight re-layouts for one core."""
    W1 = np.asarray(inputs["W1"], np.float32)
    W2 = np.asarray(inputs["W2"], np.float32)
    Wv = np.asarray(inputs["Wv"], np.float32)
    Wc = np.asarray(inputs["Wc"], np.float32)
    b1 = np.asarray(inputs["b1"], np.float32)
    b2 = np.asarray(inputs["b2"], np.float32)
    bv = np.asarray(inputs["bv"], np.float32)
    bc = np.asarray(inputs["bc"], np.float32)

    sl = slice(core * BC, (core + 1) * BC)
    ev = np.asarray(inputs["batch_event"][sl], np.int64)          # (BC, C)
    feats = np.asarray(inputs["batch_features"][sl], np.float32)  # (BC, NF)
    dists = np.asarray(inputs["batch_distances"][sl], np.float32) # (BC, 9)
    ctx = np.asarray(inputs["batch_context"][sl], np.int64)       # (BC, N, C)

    bf = ml_dtypes.bfloat16
    # W1.T with K padded 300->EP per component, zeros in the pad rows
    w1t = np.zeros((CE, H1), np.float32)
    for c in range(C):
        w1t[EP * c : EP * c + E, :] = W1[:, E * c : E * (c + 1)].T
    wvt = np.zeros((CE, 9), np.float32)
    wvt[EP * 1 : EP * 1 + E, :] = Wv.T  # predicates = component 1

    wc_full = np.zeros((128,), np.float32)
    wc_full[32 : 32 + 9] = Wc[0, 0:9]          # dist_emb block
    wc_full[64 : 64 + NF] = Wc[0, 9 : 9 + NF]  # batch_features block
    wckp = (Wc[0, NF + 9 :] * 0.01).astype(np.float32)  # kp block, 0.01 folded

    m = {
        "w1t": w1t.astype(bf),
        "w2t": np.ascontiguousarray(W2.T).astype(bf),
        "wvt": wvt.astype(bf),
        "b1d": np.ascontiguousarray(b1.reshape(4, 128).T),
        "b2d": np.ascontiguousarray(b2.reshape(2, 128).T),
        "bvd": bv.reshape(9, 1),
        "wct": wc_full.reshape(-1, 1),
        "wckp": wckp.reshape(1, NK),
        "bcd": -bc.reshape(1, 1),
        "ndsq": np.ascontiguousarray(-(dists * dists).T),
        "featT": np.ascontiguousarray(feats.T),
    }

    if fast:
        table = np.asarray(inputs["event_table"])
        allidx = np.concatenate([ctx.reshape(-1), ev.reshape(-1)])
        uniq, inv = np.unique(allidx, return_inverse=True)
        assert len(uniq) <= CT
        ctab = np.zeros((CT, EP), bf)
        ctab[: len(uniq), :E] = np.asarray(table[uniq], np.float32)
        rctx = inv[: ctx.size].astype(np.int16).reshape(BC, N, C)
        rev = inv[ctx.size :].astype(np.int16).reshape(BC, C)

        # context: per (g, s) gather of 512 idx with i = c*128 + p
        ci = rctx.reshape(GROUPS, SUBT, N, C).transpose(0, 1, 3, 2)  # g,s,c,p
        cidx = np.concatenate(
            [
                _wrap16(ci[g, s].reshape(-1))
                for g in range(GROUPS)
                for s in range(SUBT)
            ],
            axis=1,
        )
        # event: i = c*128 + b; b >= BC -> row 0 junk
        ei = np.zeros((C, 128), np.int16)
        ei[:, :BC] = rev.T
        m["ctab"] = ctab
        m["cidx"] = np.ascontiguousarray(cidx)
        m["eidx"] = np.ascontiguousarray(_wrap16(ei.reshape(-1)))
    else:
        m["table"] = np.ascontiguousarray(
            np.asarray(inputs["event_table"], np.float32)
        )
        m["ctxidx"] = np.ascontiguousarray(
            ctx.astype(np.int32).transpose(1, 0, 2).reshape(128, BC * C)
        )
        m["evidx"] = ev.astype(np.int32)
    return m


def kernel(**inputs) -> np.ndarray:
    # fast path requires every shard's unique row count to fit int16
    fast = True
    ctx = np.asarray(inputs["batch_context"], np.int64)
    ev = np.asarray(inputs["batch_event"], np.int64)
    for core in range(NCORES):
        sl = slice(core * BC, (core + 1) * BC)
        nuniq = len(np.unique(np.concatenate(
            [ctx[sl].reshape(-1), ev[sl].reshape(-1)])))
        if nuniq > CT:
            fast = False
            break
    nc = _build_program(fast)
    in_maps = [_prep_core_inputs(inputs, core, fast) for core in range(NCORES)]
    res = run_bass_kernel_spmd(nc, in_maps, core_ids=list(range(NCORES)))
    return np.concatenate([r["out"] for r in res.results], axis=0)


if __name__ == "__main__":
    nc = _build_program(True)
    print("program built ok")



# revision 4
# speedup vs baseline: 1.2338x; 1.2338x over previous
"""Trainium2 Bass kernel for nn_EventPairCompositionModel.

Strategy (data-parallel over batch, 8 cores, B=512 -> 64 per core):
  - Host compacts the 60MB f32 table per core to the ~24K unique rows its
    shard touches, stored fp8e4m3 (x16 scale) with rows padded to 512B,
    indices remapped to int16.  SWDGE dma_gather (transpose mode) fetches
    embeddings; its 16-bit transpose granularity lands fp8 element pairs
    as adjacent bytes - exactly the moving-operand layout fp8 DoubleRow
    matmuls consume (2 K-tiles per instruction, 2x PE throughput).
  - MLP1 (1200->512) and MLP2 (512->256) run fp8 DoubleRow; PSUM stays
    fp32.  MLP1's ReLU+bias+requant (x256) is a single vector-engine
    tensor_scalar (scale folded to 1 by the 16*16 weight/act scales);
    MLP2's ReLU descales by 1/4096 on the scalar engine into bf16.
  - Cosine numerators/denominators via small bf16 matmuls landing n-on-
    partitions; norms folded through one exp(-0.5 ln x); KNRM pooling via
    ones-matmul partition reductions; final linear + sigmoid on-chip.
  - If a shard touches >32767 unique rows (can't happen for random
    inputs), falls back to the original bf16 indirect-DMA path.
All 8 cores run the identical program on their own batch shard (SPMD, no
collectives); host concatenates the 8 (64,1) outputs.
"""

import numpy as np
import ml_dtypes

import concourse.bacc as bacc
import concourse.bass as bass
import concourse.tile as tile
import concourse.mybir as mybir
from concourse.bass import IndirectOffsetOnAxis
from concourse.bass_utils import run_bass_kernel_spmd
from concourse import library_config

F32 = mybir.dt.float32
BF16 = mybir.dt.bfloat16
F8 = mybir.dt.float8e4
I16 = mybir.dt.int16
I32 = mybir.dt.int32
AF = mybir.ActivationFunctionType
DR = mybir.MatmulPerfMode.DoubleRow

# Problem shapes (hardcoded per spec)
B, N, C, E = 512, 128, 4, 300
V = 50000
H1, H2 = 512, 256
NF, NK = 8, 11
NCORES = 8
BC = B // NCORES          # 64 batches per core
CT = 32768                # compact table rows (int16-indexable)
GROUPS = (BC * N) // 512  # 16 groups of 512 (b,n) pairs
SUBT = 4                  # batches per group
EB = 128                  # event-path width (64 real b + 64 junk)

# fast path (fp8)
EP8 = 512                 # fp8 elems per ctab row (300 real + pad, 512B)
KT8 = 16                  # MLP1 K-tiles: t = 4c + 2jc + b, elem = 256jc+2p+b
SA = 16.0                 # table scale
SW = 16.0                 # W1/W2/Wv scale
C1 = 256.0                # s1 requant scale (= SA*SW so act1 scale == 1)

# slow path (bf16)
EP = 384                  # padded embedding stride (768B)
CE = C * EP               # 1536
KT = CE // 128            # 12 K-tiles

MUS = [1.0, 0.9, 0.7, 0.5, 0.3, 0.1, -0.1, -0.3, -0.5, -0.7, -0.9]
SIGMAS = [1e-3] + [0.1] * 10

_PROGRAM_CACHE = {}


def _spread_swdge_queues(nc):
    # Spread SWDGE gathers across the 4 queues. The ucode locks each DMASW
    # semaphore lane to one queue, and Tile assigns lanes round-robin in
    # scheduled order, so derive queue from the assigned lane post-compile.
    import re as _re
    for blk in nc.m.functions[0].blocks:
        for inst in blk.instructions:
            if type(inst).__name__ == "InstDMAGatherAnt":
                for u in inst.sync_info.on_update:
                    m = _re.match(r"DMASW(\d+)_", u.ant_name or "")
                    if m:
                        inst.queue_num = int(m.group(1)) % 4
                        break


def _build_fast():
    nc = bacc.Bacc("TRN2", target_bir_lowering=False, debug=False, num_swdge_queues=4)

    # ---- DRAM I/O ----
    ctab = nc.dram_tensor("ctab", (CT, EP8), F8, kind="ExternalInput")
    cidx = nc.dram_tensor("cidx", (128, GROUPS * C * 32), I16, kind="ExternalInput")
    eidx = nc.dram_tensor("eidx", (128, 32), I16, kind="ExternalInput")
    w1t = nc.dram_tensor("w1t", (KT8 * 128, H1), F8, kind="ExternalInput")
    w2t = nc.dram_tensor("w2t", (H1, H2), F8, kind="ExternalInput")
    wvt = nc.dram_tensor("wvt", (4 * 128, 9), F8, kind="ExternalInput")
    b1d = nc.dram_tensor("b1d", (128, 4), F32, kind="ExternalInput")
    b2d = nc.dram_tensor("b2d", (128, 2), F32, kind="ExternalInput")
    bvd = nc.dram_tensor("bvd", (9, 1), F32, kind="ExternalInput")
    wct = nc.dram_tensor("wct", (128, 1), F32, kind="ExternalInput")
    wckp = nc.dram_tensor("wckp", (1, NK), F32, kind="ExternalInput")
    bcd = nc.dram_tensor("bcd", (1, 1), F32, kind="ExternalInput")
    ndsq = nc.dram_tensor("ndsq", (9, BC), F32, kind="ExternalInput")
    featT = nc.dram_tensor("featT", (NF, BC), F32, kind="ExternalInput")
    out_d = nc.dram_tensor("out", (BC, 1), F32, kind="ExternalOutput")

    with tile.TileContext(nc) as tc:
        with (
            tc.tile_pool(name="consts", bufs=1) as cpool,
            tc.tile_pool(name="xt", bufs=4) as xtpool,
            tc.tile_pool(name="s1", bufs=3) as s1pool,
            tc.tile_pool(name="s2", bufs=4) as s2pool,
            tc.tile_pool(name="csq", bufs=4) as csqpool,
            tc.tile_pool(name="small", bufs=2) as smpool,
            tc.tile_pool(name="pm1", bufs=2, space="PSUM") as pm1,
            tc.tile_pool(name="pm2", bufs=2, space="PSUM") as pm2,
            tc.tile_pool(name="ptn", bufs=1, space="PSUM") as ptn,
            tc.tile_pool(name="pmisc", bufs=2, space="PSUM") as pmisc,
        ):
            # ---- load constants ----
            nc.gpsimd.load_library(library_config.mlp)
            cidx_s = cpool.tile([128, GROUPS * C * 32], I16)
            nc.sync.dma_start(cidx_s[:], cidx.ap())
            eidx_s = cpool.tile([128, 32], I16)
            nc.sync.dma_start(eidx_s[:], eidx.ap())
            w1t_s = cpool.tile([128, KT8 * H1], F8)
            nc.sync.dma_start(
                w1t_s[:].rearrange("p (t m) -> p t m", t=KT8),
                w1t.ap().rearrange("(t p) m -> p t m", p=128),
            )
            w2t_s = cpool.tile([128, 4 * H2], F8)
            nc.scalar.dma_start(
                w2t_s[:].rearrange("p (t m) -> p t m", t=4),
                w2t.ap().rearrange("(t p) m -> p t m", p=128),
            )
            wvt_s = cpool.tile([128, 4 * 9], F8)
            nc.scalar.dma_start(
                wvt_s[:].rearrange("p (t m) -> p t m", t=4),
                wvt.ap().rearrange("(t p) m -> p t m", p=128),
            )
            b1_s = cpool.tile([128, 4], F32)
            nc.sync.dma_start(b1_s[:], b1d.ap())
            b2_s = cpool.tile([128, 2], F32)
            nc.sync.dma_start(b2_s[:], b2d.ap())
            bv_s = cpool.tile([9, 1], F32)
            nc.sync.dma_start(bv_s[:], bvd.ap())
            wct_s = cpool.tile([128, 1], F32)
            nc.sync.dma_start(wct_s[:], wct.ap())
            wckp_s = cpool.tile([1, NK], F32)
            nc.sync.dma_start(wckp_s[:], wckp.ap())
            bc_s = cpool.tile([1, 1], F32)
            nc.sync.dma_start(bc_s[:], bcd.ap())
            ndsq_s = cpool.tile([9, BC], F32)
            nc.sync.dma_start(ndsq_s[:], ndsq.ap())
            feat_s = cpool.tile([128, BC], F32)
            nc.vector.memset(feat_s[:], 0.0)
            nc.sync.dma_start(feat_s[64 : 64 + NF, :], featT.ap())
            ones_s = cpool.tile([128, 1], BF16)
            nc.vector.memset(ones_s[:], 1.0)
            onesrow_s = cpool.tile([1, 128], F32)
            nc.vector.memset(onesrow_s[:], 1.0)
            onesf_s = cpool.tile([128, 1], F32)
            nc.vector.memset(onesf_s[:], 1.0)
            eps_s = cpool.tile([128, 1], F32)
            nc.vector.memset(eps_s[:], 1e-20)
            mub_s = cpool.tile([128, NK], F32)
            for k in range(NK):
                nc.vector.memset(mub_s[:, k : k + 1], -MUS[k])

            # views: pair u = t//2, "two" = the byte dim consumed by DoubleRow
            w1_v = w1t_s[:].rearrange("p (u two m) -> p u two m", u=KT8 // 2, two=2)
            w2_v = w2t_s[:].rearrange("p (q two m) -> p q two m", q=2, two=2)
            wv_v = wvt_s[:].rearrange("p (u two v) -> p u two v", u=2, two=2)

            # ---- event path (EB=128 lanes, only 0..63 meaningful) ----
            xeT = cpool.tile([128, (EP8 // 128) * 512], F8)  # 2048/partition
            nc.gpsimd.dma_gather(
                out_ap=xeT[:].rearrange("p (j i) -> p j i", j=EP8 // 128),
                in_ap=ctab.ap(),
                idxs_ap=eidx_s[:],
                num_idxs=512,
                num_idxs_reg=512,
                elem_size=EP8,
                transpose=True,
            )
            # flat = 1024*jc + 256*c + 2*eb + b; elem = 256*jc + 2p + b
            xeT_v = xeT[:].rearrange("p (j c i b) -> p j c b i", j=2, c=C, b=2)

            s1e = cpool.tile([128, 4 * EB], F8)
            for m in range(4):
                pe = pmisc.tile([128, EB], F32, tag="pmisc", name="pe")
                k = 0
                for c in range(C):
                    for jc in range(2):
                        nc.tensor.matmul(
                            pe[:],
                            w1_v[:, 2 * c + jc, :, 128 * m : 128 * m + 128],
                            xeT_v[:, jc, c],
                            start=(k == 0),
                            stop=(k == 7),
                            perf_mode=DR,
                        )
                        k += 1
                nc.vector.tensor_scalar(
                    s1e[:, EB * m : EB * (m + 1)], pe[:],
                    b1_s[:, m : m + 1], 0.0,
                    mybir.AluOpType.add, mybir.AluOpType.max,
                )

            s1e_v = s1e[:].rearrange("p (q two i) -> p q two i", q=2, two=2)
            eh2 = [
                cpool.tile([128, EB], BF16, tag=f"eh2_{k}", name=f"eh2_{k}")
                for k in range(2)
            ]
            for mm in range(2):
                pe2 = pmisc.tile([128, EB], F32, tag="pmisc", name="pe2")
                for q in range(2):
                    nc.tensor.matmul(
                        pe2[:],
                        w2_v[:, q, :, 128 * mm : 128 * mm + 128],
                        s1e_v[:, q],
                        start=(q == 0),
                        stop=(q == 1),
                        perf_mode=DR,
                    )
                nc.scalar.activation(
                    eh2[mm][:], pe2[:], AF.Relu,
                    bias=b2_s[:, mm : mm + 1], scale=1.0 / (SW * C1),
                )

            # variances -> dist_emb rows 32..40 of feat_s (regular fp8
            # matmuls: DoubleRow rejects 9-wide stationaries)
            pv = pmisc.tile([9, EB], F32, tag="pmisc", name="pv")
            for tp in range(4):
                jc, bb = tp // 2, tp % 2
                nc.tensor.matmul(
                    pv[:], wvt_s[:, 9 * tp : 9 * (tp + 1)],
                    xeT_v[:, jc, 1, bb, :],
                    start=(tp == 0), stop=(tp == 3),
                )
            ez_s = smpool.tile([9, EB], F32)
            nc.scalar.activation(
                ez_s[:], pv[:], AF.Exp, bias=bv_s[:], scale=1.0 / (SA * SW)
            )
            ez1_s = smpool.tile([9, EB], F32)
            nc.vector.tensor_scalar_add(ez1_s[:], ez_s[:], 1.0)
            var_s = smpool.tile([9, EB], F32)
            nc.scalar.activation(var_s[:], ez1_s[:], AF.Ln)
            rv_s = smpool.tile([9, EB], F32)
            nc.vector.reciprocal(rv_s[:], var_s[:])
            q_s = smpool.tile([9, BC], F32)
            nc.vector.tensor_mul(q_s[:], ndsq_s[:], rv_s[:, 0:BC])
            nc.scalar.activation(feat_s[32:41, :], q_s[:], AF.Exp)

            # |e|^2 per b, broadcast to all 128 partitions via outer product
            esq = [
                smpool.tile([128, EB], BF16, tag=f"esq_{k}", name=f"esq_{k}")
                for k in range(2)
            ]
            for k in range(2):
                nc.vector.tensor_mul(esq[k][:], eh2[k][:], eh2[k][:])
            pne = pmisc.tile([1, EB], F32, tag="pmisc", name="pne")
            for k in range(2):
                nc.tensor.matmul(
                    pne[:], ones_s[:], esq[k][:], start=(k == 0), stop=(k == 1)
                )
            ne2_s = smpool.tile([1, BC], F32)
            nc.scalar.copy(ne2_s[:], pne[:, 0:BC])
            pne2bc = pmisc.tile([128, BC], F32, tag="pmisc", name="pne2bc")
            nc.tensor.matmul(
                pne2bc[:], onesrow_s[:], ne2_s[:], start=True, stop=True
            )
            ne2bc_s = cpool.tile([128, BC], F32)
            nc.scalar.copy(ne2bc_s[:], pne2bc[:])

            # persistent SBUF accumulators, n on partitions, b on free
            traw_s = cpool.tile([128, BC], F32)
            ncsq_s = cpool.tile([128, BC], F32)

            # ---- context groups ----
            for g in range(GROUPS):
                xt = xtpool.tile([128, C * 2048], F8)
                for c in range(C):
                    nc.gpsimd.dma_gather(
                        out_ap=xt[:, 2048 * c : 2048 * (c + 1)].rearrange(
                            "p (j i) -> p j i", j=EP8 // 128
                        ),
                        in_ap=ctab.ap(),
                        idxs_ap=cidx_s[:, 32 * (C * g + c) : 32 * (C * g + c + 1)],
                        num_idxs=512,
                        num_idxs_reg=512,
                        elem_size=EP8,
                        transpose=True,
                    )
                # flat = 2048c + 1024jc + 2i + b, i = z*128 + n
                xt_v = xt[:].rearrange("p (c j i b) -> p c j b i", c=C, j=2, b=2)

                s1 = s1pool.tile([128, 4 * 512], F8, tag="s1", name="s1")
                for m in range(4):
                    p1 = pm1.tile([128, 512], F32)
                    k = 0
                    for c in range(C):
                        for jc in range(2):
                            nc.tensor.matmul(
                                p1[:],
                                w1_v[:, 2 * c + jc, :, 128 * m : 128 * m + 128],
                                xt_v[:, c, jc],
                                start=(k == 0),
                                stop=(k == 7),
                                perf_mode=DR,
                            )
                            k += 1
                    nc.vector.tensor_scalar(
                        s1[:, 512 * m : 512 * (m + 1)], p1[:],
                        b1_s[:, m : m + 1], 0.0,
                        mybir.AluOpType.add, mybir.AluOpType.max,
                    )

                s1_v = s1[:].rearrange("p (q two i) -> p q two i", q=2, two=2)
                s2 = [
                    s2pool.tile([128, 512], BF16, tag=f"s2_{m}", name=f"s2_{m}")
                    for m in range(2)
                ]
                for mm in range(2):
                    p2 = pm2.tile([128, 512], F32)
                    for q in range(2):
                        nc.tensor.matmul(
                            p2[:],
                            w2_v[:, q, :, 128 * mm : 128 * mm + 128],
                            s1_v[:, q],
                            start=(q == 0),
                            stop=(q == 1),
                            perf_mode=DR,
                        )
                    nc.scalar.activation(
                        s2[mm][:], p2[:], AF.Relu,
                        bias=b2_s[:, mm : mm + 1], scale=1.0 / (SW * C1),
                    )

                csq = [
                    csqpool.tile([128, 512], BF16, tag=f"csq_{m}", name=f"csq_{m}")
                    for m in range(2)
                ]
                for m in range(2):
                    nc.vector.tensor_mul(csq[m][:], s2[m][:], s2[m][:])

                # raw dots and |c|^2, n on partitions, one column per b
                pT = ptn.tile([128, SUBT], F32, tag="pT", name="pT")
                pN = ptn.tile([128, SUBT], F32, tag="pN", name="pN")
                for s in range(SUBT):
                    b = SUBT * g + s
                    for k in range(2):
                        nc.tensor.matmul(
                            pT[:, s : s + 1],
                            s2[k][:, 128 * s : 128 * (s + 1)],
                            eh2[k][:, b : b + 1],
                            start=(k == 0),
                            stop=(k == 1),
                        )
                    for k in range(2):
                        nc.tensor.matmul(
                            pN[:, s : s + 1],
                            csq[k][:, 128 * s : 128 * (s + 1)],
                            ones_s[:],
                            start=(k == 0),
                            stop=(k == 1),
                        )
                nc.scalar.copy(traw_s[:, SUBT * g : SUBT * (g + 1)], pT[:])
                nc.scalar.copy(ncsq_s[:, SUBT * g : SUBT * (g + 1)], pN[:])

            # ---- kernel pooling (tiles are [n=128, b=64]) ----
            prodn = smpool.tile([128, BC], F32, tag="prodn")
            nc.vector.tensor_mul(prodn[:], ncsq_s[:], ne2bc_s[:])
            lnp = smpool.tile([128, BC], F32, tag="lnp")
            nc.scalar.activation(lnp[:], prodn[:], AF.Ln, bias=eps_s[:])
            nrmf = smpool.tile([128, BC], F32, tag="nrmf")
            nc.scalar.activation(nrmf[:], lnp[:], AF.Exp, scale=-0.5)
            trans = cpool.tile([128, BC], F32)
            nc.vector.tensor_mul(trans[:], traw_s[:], nrmf[:])

            kpp_s = cpool.tile([1, NK * BC], F32)
            for k in range(NK):
                sq = smpool.tile([128, BC], F32, tag="sq", name="sq")
                nc.scalar.activation(
                    sq[:], trans[:], AF.Square, bias=mub_s[:, k : k + 1]
                )
                arg = smpool.tile([128, BC], F32, tag="arg", name="arg")
                nc.vector.tensor_scalar(
                    arg[:], sq[:],
                    -1.0 / (2.0 * SIGMAS[k] ** 2), -87.0,
                    mybir.AluOpType.mult, mybir.AluOpType.max,
                )
                ek = smpool.tile([128, BC], F32, tag="ek", name="ek")
                nc.scalar.activation(ek[:], arg[:], AF.Exp)
                pp = pmisc.tile([1, BC], F32, tag="pmisc", name="pp")
                nc.tensor.matmul(pp[:], onesf_s[:], ek[:], start=True, stop=True)
                nc.scalar.copy(kpp_s[:, BC * k : BC * (k + 1)], pp[:])

            kpc_s = smpool.tile([1, NK * BC], F32, tag="kpc")
            nc.vector.tensor_scalar_max(kpc_s[:], kpp_s[:], 1e-10)
            kpl_s = smpool.tile([1, NK * BC], F32, tag="kpl")
            nc.scalar.activation(kpl_s[:], kpc_s[:], AF.Ln)

            # weighted sum over k: kps[b] = sum_k wckp[k] * kpl[k, b]
            kpw_s = smpool.tile([1, BC * NK], F32, tag="kpw")
            kpl_v = kpl_s[:].rearrange("p (k b) -> p b k", k=NK)
            wck_v = wckp_s[:][:, None, :].broadcast_to([1, BC, NK])
            kpw_v = kpw_s[:].rearrange("p (b k) -> p b k", b=BC)
            nc.vector.tensor_tensor(
                out=kpw_v, in0=kpl_v, in1=wck_v, op=mybir.AluOpType.mult
            )
            kps_s = smpool.tile([1, BC], F32, tag="kps")
            nc.vector.reduce_sum(
                out=kps_s[:], in_=kpw_v, axis=mybir.AxisListType.X
            )

            # ---- final score ----
            psc = pmisc.tile([1, BC], F32, tag="pmisc", name="psc")
            nc.tensor.matmul(psc[:], wct_s[:], feat_s[:], start=True, stop=True)
            tot_s = smpool.tile([1, BC], F32, tag="tot")
            nc.vector.tensor_add(tot_s[:], psc[:], kps_s[:])
            emx = smpool.tile([1, BC], F32, tag="emx")
            nc.scalar.activation(emx[:], tot_s[:], AF.Exp, bias=bc_s[:], scale=-1.0)
            emx1 = smpool.tile([1, BC], F32, tag="emx1")
            nc.vector.tensor_scalar_add(emx1[:], emx[:], 1.0)
            outs = smpool.tile([1, BC], F32, tag="outs")
            nc.vector.reciprocal(outs[:], emx1[:])
            nc.sync.dma_start(out_d.ap().rearrange("b one -> one b"), outs[:])

    nc.compile()
    _spread_swdge_queues(nc)
    return nc


def _build_slow():
    """Original bf16 fallback: indirect-DMA gather of the full f32 table."""
    nc = bacc.Bacc("TRN2", target_bir_lowering=False, debug=False, num_swdge_queues=4)

    ctab = nc.dram_tensor("table", (V + 1, E), F32, kind="ExternalInput")
    cidx = nc.dram_tensor("ctxidx", (128, BC * C), I32, kind="ExternalInput")
    eidx = nc.dram_tensor("evidx", (BC, C), I32, kind="ExternalInput")
    w1t = nc.dram_tensor("w1t", (CE, H1), BF16, kind="ExternalInput")
    w2t = nc.dram_tensor("w2t", (H1, H2), BF16, kind="ExternalInput")
    wvt = nc.dram_tensor("wvt", (CE, 9), BF16, kind="ExternalInput")
    b1d = nc.dram_tensor("b1d", (128, 4), F32, kind="ExternalInput")
    b2d = nc.dram_tensor("b2d", (128, 2), F32, kind="ExternalInput")
    bvd = nc.dram_tensor("bvd", (9, 1), F32, kind="ExternalInput")
    wct = nc.dram_tensor("wct", (128, 1), F32, kind="ExternalInput")
    wckp = nc.dram_tensor("wckp", (1, NK), F32, kind="ExternalInput")
    bcd = nc.dram_tensor("bcd", (1, 1), F32, kind="ExternalInput")
    ndsq = nc.dram_tensor("ndsq", (9, BC), F32, kind="ExternalInput")
    featT = nc.dram_tensor("featT", (NF, BC), F32, kind="ExternalInput")
    out_d = nc.dram_tensor("out", (BC, 1), F32, kind="ExternalOutput")

    with tile.TileContext(nc) as tc:
        with (
            tc.tile_pool(name="consts", bufs=1) as cpool,
            tc.tile_pool(name="xg", bufs=4) as xgpool,
            tc.tile_pool(name="xt", bufs=4) as xtpool,
            tc.tile_pool(name="s1", bufs=8) as s1pool,
            tc.tile_pool(name="s2", bufs=4) as s2pool,
            tc.tile_pool(name="csq", bufs=4) as csqpool,
            tc.tile_pool(name="small", bufs=2) as smpool,
            tc.tile_pool(name="pm1", bufs=2, space="PSUM") as pm1,
            tc.tile_pool(name="pm2", bufs=2, space="PSUM") as pm2,
            tc.tile_pool(name="ptn", bufs=1, space="PSUM") as ptn,
            tc.tile_pool(name="pmisc", bufs=2, space="PSUM") as pmisc,
        ):
            w1t_s = cpool.tile([128, KT * H1], BF16)
            nc.sync.dma_start(
                w1t_s[:].rearrange("p (t m) -> p t m", t=KT),
                w1t.ap().rearrange("(t p) m -> p t m", p=128),
            )
            w2t_s = cpool.tile([128, 4 * H2], BF16)
            nc.scalar.dma_start(
                w2t_s[:].rearrange("p (t m) -> p t m", t=4),
                w2t.ap().rearrange("(t p) m -> p t m", p=128),
            )
            wvt_s = cpool.tile([128, KT * 9], BF16)
            nc.scalar.dma_start(
                wvt_s[:].rearrange("p (t m) -> p t m", t=KT),
                wvt.ap().rearrange("(t p) m -> p t m", p=128),
            )
            b1_s = cpool.tile([128, 4], F32)
            nc.sync.dma_start(b1_s[:], b1d.ap())
            b2_s = cpool.tile([128, 2], F32)
            nc.sync.dma_start(b2_s[:], b2d.ap())
            bv_s = cpool.tile([9, 1], F32)
            nc.sync.dma_start(bv_s[:], bvd.ap())
            wct_s = cpool.tile([128, 1], F32)
            nc.sync.dma_start(wct_s[:], wct.ap())
            wckp_s = cpool.tile([1, NK], F32)
            nc.sync.dma_start(wckp_s[:], wckp.ap())
            bc_s = cpool.tile([1, 1], F32)
            nc.sync.dma_start(bc_s[:], bcd.ap())
            cidx_s = cpool.tile([128, BC * C], I32)
            nc.sync.dma_start(cidx_s[:], cidx.ap())
            eidx_s = cpool.tile([BC, C], I32)
            nc.sync.dma_start(eidx_s[:], eidx.ap())
            ndsq_s = cpool.tile([9, BC], F32)
            nc.sync.dma_start(ndsq_s[:], ndsq.ap())
            feat_s = cpool.tile([128, BC], F32)
            nc.vector.memset(feat_s[:], 0.0)
            nc.sync.dma_start(feat_s[64 : 64 + NF, :], featT.ap())
            ones_s = cpool.tile([128, 1], BF16)
            nc.vector.memset(ones_s[:], 1.0)
            onesrow_s = cpool.tile([1, 128], F32)
            nc.vector.memset(onesrow_s[:], 1.0)
            onesf_s = cpool.tile([128, 1], F32)
            nc.vector.memset(onesf_s[:], 1.0)
            eps_s = cpool.tile([128, 1], F32)
            nc.vector.memset(eps_s[:], 1e-20)
            mub_s = cpool.tile([128, NK], F32)
            for k in range(NK):
                nc.vector.memset(mub_s[:, k : k + 1], -MUS[k])

            # ---- event path ----
            xeT = cpool.tile([128, KT * EB], BF16)
            xe = cpool.tile([EB, CE], BF16)
            nc.vector.memset(xe[:], 0.0)
            nc.gpsimd.indirect_dma_start(
                out=xe[0:BC, :].rearrange("p (c e) -> p c e", c=C)[:, :, 0:E],
                out_offset=None,
                in_=ctab.ap(),
                in_offset=IndirectOffsetOnAxis(ap=eidx_s[:], axis=0),
            )
            nc.sync.dma_start_transpose(
                xeT[:].rearrange("p (j i) -> p j i", j=KT), xe[:]
            )

            def xeT_k(j):
                return xeT[:, EB * j : EB * (j + 1)]

            s1e = cpool.tile([128, 4 * EB], BF16)
            for m in range(4):
                pe = pmisc.tile([128, EB], F32, tag="pmisc", name="pe")
                for j in range(KT):
                    nc.tensor.matmul(
                        pe[:],
                        w1t_s[:, H1 * j + 128 * m : H1 * j + 128 * m + 128],
                        xeT_k(j),
                        start=(j == 0),
                        stop=(j == KT - 1),
                    )
                nc.scalar.activation(
                    s1e[:, EB * m : EB * (m + 1)], pe[:], AF.Relu,
                    bias=b1_s[:, m : m + 1],
                )

            eh2 = [
                cpool.tile([128, EB], BF16, tag=f"eh2_{k}", name=f"eh2_{k}")
                for k in range(2)
            ]
            for m in range(2):
                pe2 = pmisc.tile([128, EB], F32, tag="pmisc", name="pe2")
                for j in range(4):
                    nc.tensor.matmul(
                        pe2[:],
                        w2t_s[:, H2 * j + 128 * m : H2 * j + 128 * m + 128],
                        s1e[:, EB * j : EB * (j + 1)],
                        start=(j == 0),
                        stop=(j == 3),
                    )
                nc.scalar.activation(
                    eh2[m][:], pe2[:], AF.Relu, bias=b2_s[:, m : m + 1]
                )

            pv = pmisc.tile([9, EB], F32, tag="pmisc", name="pv")
            for j in range(KT):
                nc.tensor.matmul(
                    pv[:],
                    wvt_s[:, 9 * j : 9 * (j + 1)],
                    xeT_k(j),
                    start=(j == 0),
                    stop=(j == KT - 1),
                )
            ez_s = smpool.tile([9, EB], F32)
            nc.scalar.activation(ez_s[:], pv[:], AF.Exp, bias=bv_s[:])
            ez1_s = smpool.tile([9, EB], F32)
            nc.vector.tensor_scalar_add(ez1_s[:], ez_s[:], 1.0)
            var_s = smpool.tile([9, EB], F32)
            nc.scalar.activation(var_s[:], ez1_s[:], AF.Ln)
            rv_s = smpool.tile([9, EB], F32)
            nc.vector.reciprocal(rv_s[:], var_s[:])
            q_s = smpool.tile([9, BC], F32)
            nc.vector.tensor_mul(q_s[:], ndsq_s[:], rv_s[:, 0:BC])
            nc.scalar.activation(feat_s[32:41, :], q_s[:], AF.Exp)

            esq = [
                smpool.tile([128, EB], BF16, tag=f"esq_{k}", name=f"esq_{k}")
                for k in range(2)
            ]
            for k in range(2):
                nc.vector.tensor_mul(esq[k][:], eh2[k][:], eh2[k][:])
            pne = pmisc.tile([1, EB], F32, tag="pmisc", name="pne")
            for k in range(2):
                nc.tensor.matmul(
                    pne[:], ones_s[:], esq[k][:], start=(k == 0), stop=(k == 1)
                )
            ne2_s = smpool.tile([1, BC], F32)
            nc.scalar.copy(ne2_s[:], pne[:, 0:BC])
            pne2bc = pmisc.tile([128, BC], F32, tag="pmisc", name="pne2bc")
            nc.tensor.matmul(
                pne2bc[:], onesrow_s[:], ne2_s[:], start=True, stop=True
            )
            ne2bc_s = cpool.tile([128, BC], F32)
            nc.scalar.copy(ne2bc_s[:], pne2bc[:])

            traw_s = cpool.tile([128, BC], F32)
            ncsq_s = cpool.tile([128, BC], F32)

            for g in range(GROUPS):
                xt = xtpool.tile([128, KT * 512], BF16)
                xg = xgpool.tile([128, SUBT * CE], BF16)
                nc.vector.memset(
                    xg[:].rearrange("p (q e) -> p q e", e=EP)[:, :, E:EP],
                    0.0,
                )
                for s in range(SUBT):
                    nc.gpsimd.indirect_dma_start(
                        out=xg[:]
                        .rearrange("p (q c e) -> p q c e", q=SUBT, c=C)[
                            :, s, :, 0:E
                        ],
                        out_offset=None,
                        in_=ctab.ap(),
                        in_offset=IndirectOffsetOnAxis(
                            ap=cidx_s[
                                :, (SUBT * g + s) * C : (SUBT * g + s + 1) * C
                            ],
                            axis=0,
                        ),
                    )
                for s in range(SUBT):
                    nc.sync.dma_start_transpose(
                        xt[:].rearrange(
                            "p (j z i) -> p j z i", j=KT, z=SUBT
                        )[:, :, s, :],
                        xg[:, CE * s : CE * (s + 1)],
                    )

                def xt_k(j):
                    return xt[:, 512 * j : 512 * (j + 1)]

                s1 = [
                    s1pool.tile([128, 512], BF16, tag=f"s1_{m}", name=f"s1_{m}")
                    for m in range(4)
                ]
                for m in range(4):
                    p1 = pm1.tile([128, 512], F32)
                    for j in range(KT):
                        nc.tensor.matmul(
                            p1[:],
                            w1t_s[:, H1 * j + 128 * m : H1 * j + 128 * m + 128],
                            xt_k(j),
                            start=(j == 0),
                            stop=(j == KT - 1),
                        )
                    nc.scalar.activation(
                        s1[m][:], p1[:], AF.Relu, bias=b1_s[:, m : m + 1]
                    )

                s2 = [
                    s2pool.tile([128, 512], BF16, tag=f"s2_{m}", name=f"s2_{m}")
                    for m in range(2)
                ]
                for m in range(2):
                    p2 = pm2.tile([128, 512], F32)
                    for j in range(4):
                        nc.tensor.matmul(
                            p2[:],
                            w2t_s[:, H2 * j + 128 * m : H2 * j + 128 * m + 128],
                            s1[j][:],
                            start=(j == 0),
                            stop=(j == 3),
                        )
                    nc.scalar.activation(
                        s2[m][:], p2[:], AF.Relu, bias=b2_s[:, m : m + 1]
                    )

                csq = [
                    csqpool.tile([128, 512], BF16, tag=f"csq_{m}", name=f"csq_{m}")
                    for m in range(2)
                ]
                for m in range(2):
                    nc.vector.tensor_mul(csq[m][:], s2[m][:], s2[m][:])

                pT = ptn.tile([128, SUBT], F32, tag="pT", name="pT")
                pN = ptn.tile([128, SUBT], F32, tag="pN", name="pN")
                for s in range(SUBT):
                    b = SUBT * g + s
                    for k in range(2):
                        nc.tensor.matmul(
                            pT[:, s : s + 1],
                            s2[k][:, 128 * s : 128 * (s + 1)],
                            eh2[k][:, b : b + 1],
                            start=(k == 0),
                            stop=(k == 1),
                        )
                    for k in range(2):
                        nc.tensor.matmul(
                            pN[:, s : s + 1],
                            csq[k][:, 128 * s : 128 * (s + 1)],
                            ones_s[:],
                            start=(k == 0),
                            stop=(k == 1),
                        )
                nc.scalar.copy(traw_s[:, SUBT * g : SUBT * (g + 1)], pT[:])
                nc.scalar.copy(ncsq_s[:, SUBT * g : SUBT * (g + 1)], pN[:])

            prodn = smpool.tile([128, BC], F32, tag="prodn")
            nc.vector.tensor_mul(prodn[:], ncsq_s[:], ne2bc_s[:])
            lnp = smpool.tile([128, BC], F32, tag="lnp")
            nc.scalar.activation(lnp[:], prodn[:], AF.Ln, bias=eps_s[:])
            nrmf = smpool.tile([128, BC], F32, tag="nrmf")
            nc.scalar.activation(nrmf[:], lnp[:], AF.Exp, scale=-0.5)
            trans = cpool.tile([128, BC], F32)
            nc.vector.tensor_mul(trans[:], traw_s[:], nrmf[:])

            kpp_s = cpool.tile([1, NK * BC], F32)
            for k in range(NK):
                sq = smpool.tile([128, BC], F32, tag="sq", name="sq")
                nc.scalar.activation(
                    sq[:], trans[:], AF.Square, bias=mub_s[:, k : k + 1]
                )
                arg = smpool.tile([128, BC], F32, tag="arg", name="arg")
                nc.vector.tensor_scalar(
                    arg[:], sq[:],
                    -1.0 / (2.0 * SIGMAS[k] ** 2), -87.0,
                    mybir.AluOpType.mult, mybir.AluOpType.max,
                )
                ek = smpool.tile([128, BC], F32, tag="ek", name="ek")
                nc.scalar.activation(ek[:], arg[:], AF.Exp)
                pp = pmisc.tile([1, BC], F32, tag="pmisc", name="pp")
                nc.tensor.matmul(pp[:], onesf_s[:], ek[:], start=True, stop=True)
                nc.scalar.copy(kpp_s[:, BC * k : BC * (k + 1)], pp[:])

            kpc_s = smpool.tile([1, NK * BC], F32, tag="kpc")
            nc.vector.tensor_scalar_max(kpc_s[:], kpp_s[:], 1e-10)
            kpl_s = smpool.tile([1, NK * BC], F32, tag="kpl")
            nc.scalar.activation(kpl_s[:], kpc_s[:], AF.Ln)

            kpw_s = smpool.tile([1, BC * NK], F32, tag="kpw")
            kpl_v = kpl_s[:].rearrange("p (k b) -> p b k", k=NK)
            wck_v = wckp_s[:][:, None, :].broadcast_to([1, BC, NK])
            kpw_v = kpw_s[:].rearrange("p (b k) -> p b k", b=BC)
            nc.vector.tensor_tensor(
                out=kpw_v, in0=kpl_v, in1=wck_v, op=mybir.AluOpType.mult
            )
            kps_s = smpool.tile([1, BC], F32, tag="kps")
            nc.vector.reduce_sum(
                out=kps_s[:], in_=kpw_v, axis=mybir.AxisListType.X
            )

            psc = pmisc.tile([1, BC], F32, tag="pmisc", name="psc")
            nc.tensor.matmul(psc[:], wct_s[:], feat_s[:], start=True, stop=True)
            tot_s = smpool.tile([1, BC], F32, tag="tot")
            nc.vector.tensor_add(tot_s[:], psc[:], kps_s[:])
            emx = smpool.tile([1, BC], F32, tag="emx")
            nc.scalar.activation(emx[:], tot_s[:], AF.Exp, bias=bc_s[:], scale=-1.0)
            emx1 = smpool.tile([1, BC], F32, tag="emx1")
            nc.vector.tensor_scalar_add(emx1[:], emx[:], 1.0)
            outs = smpool.tile([1, BC], F32, tag="outs")
            nc.vector.reciprocal(outs[:], emx1[:])
            nc.sync.dma_start(out_d.ap().rearrange("b one -> one b"), outs[:])

    nc.compile()
    return nc


def _build_program(fast: bool):
    if fast in _PROGRAM_CACHE:
        return _PROGRAM_CACHE[fast]
    nc = _build_fast() if fast else _build_slow()
    _PROGRAM_CACHE[fast] = nc
    return nc


def _wrap16(flat_idx):
    """int16 index list -> (128, n/16) tile layout: unwrapped[i] =
    tile[i % 16, i // 16], replicated into all 8 16-partition stripes."""
    n = flat_idx.shape[0]
    t = np.zeros((16, n // 16), np.int16)
    t[np.arange(n) % 16, np.arange(n) // 16] = flat_idx
    return np.tile(t, (8, 1))


def _prep_core_inputs(inputs, core, fast):
    """Host-side shard + weight re-layouts for one core."""
    W1 = np.asarray(inputs["W1"], np.float32)
    W2 = np.asarray(inputs["W2"], np.float32)
    Wv = np.asarray(inputs["Wv"], np.float32)
    Wc = np.asarray(inputs["Wc"], np.float32)
    b1 = np.asarray(inputs["b1"], np.float32)
    b2 = np.asarray(inputs["b2"], np.float32)
    bv = np.asarray(inputs["bv"], np.float32)
    bc = np.asarray(inputs["bc"], np.float32)

    sl = slice(core * BC, (core + 1) * BC)
    ev = np.asarray(inputs["batch_event"][sl], np.int64)          # (BC, C)
    feats = np.asarray(inputs["batch_features"][sl], np.float32)  # (BC, NF)
    dists = np.asarray(inputs["batch_distances"][sl], np.float32) # (BC, 9)
    ctx = np.asarray(inputs["batch_context"][sl], np.int64)       # (BC, N, C)

    wc_full = np.zeros((128,), np.float32)
    wc_full[32 : 32 + 9] = Wc[0, 0:9]          # dist_emb block
    wc_full[64 : 64 + NF] = Wc[0, 9 : 9 + NF]  # batch_features block
    wckp = (Wc[0, NF + 9 :] * 0.01).astype(np.float32)  # kp block, 0.01 folded

    m = {
        "bvd": bv.reshape(9, 1),
        "wct": wc_full.reshape(-1, 1),
        "wckp": wckp.reshape(1, NK),
        "bcd": -bc.reshape(1, 1),
        "ndsq": np.ascontiguousarray(-(dists * dists).T),
        "featT": np.ascontiguousarray(feats.T),
        "b2d": np.ascontiguousarray(b2.reshape(2, 128).T),
    }

    if fast:
        f8 = ml_dtypes.float8_e4m3
        p = np.arange(128)
        # W1.T in the transpose-gather's K-permutation: K-tile t = 4c+2jc+b
        # holds elements e = 256*jc + 2p + b of component c, x16 scale
        w1t = np.zeros((KT8 * 128, H1), np.float32)
        wvt = np.zeros((4 * 128, 9), np.float32)
        for jc in range(2):
            for b in range(2):
                e = 256 * jc + 2 * p + b
                msk = e < E
                for c in range(C):
                    t = 4 * c + 2 * jc + b
                    w1t[128 * t + p[msk], :] = W1[:, E * c + e[msk]].T
                tp = 2 * jc + b
                wvt[128 * tp + p[msk], :] = Wv[:, e[msk]].T
        m["w1t"] = (w1t * SW).astype(f8)
        m["wvt"] = (wvt * SW).astype(f8)
        m["w2t"] = (np.ascontiguousarray(W2.T) * SW).astype(f8)
        m["b1d"] = np.ascontiguousarray((b1 * C1).reshape(4, 128).T)

        table = np.asarray(inputs["event_table"])
        allidx = np.concatenate([ctx.reshape(-1), ev.reshape(-1)])
        uniq, inv = np.unique(allidx, return_inverse=True)
        assert len(uniq) <= CT
        ctab = np.zeros((CT, EP8), f8)
        ctab[: len(uniq), :E] = (np.asarray(table[uniq], np.float32) * SA).astype(f8)
        rctx = inv[: ctx.size].astype(np.int16).reshape(BC, N, C)
        rev = inv[ctx.size :].astype(np.int16).reshape(BC, C)

        # context: per (g, c) gather of 512 idx, order i = z*128 + n
        ci = rctx.reshape(GROUPS, SUBT, N, C).transpose(0, 3, 1, 2)  # g,c,z,n
        cidx = np.concatenate(
            [
                _wrap16(ci[g, c].reshape(-1))
                for g in range(GROUPS)
                for c in range(C)
            ],
            axis=1,
        )
        # event: i = c*128 + b; b >= BC -> row 0 junk
        ei = np.zeros((C, 128), np.int16)
        ei[:, :BC] = rev.T
        m["ctab"] = ctab
        m["cidx"] = np.ascontiguousarray(cidx)
        m["eidx"] = np.ascontiguousarray(_wrap16(ei.reshape(-1)))
    else:
        bf = ml_dtypes.bfloat16
        w1t = np.zeros((CE, H1), np.float32)
        for c in range(C):
            w1t[EP * c : EP * c + E, :] = W1[:, E * c : E * (c + 1)].T
        wvt = np.zeros((CE, 9), np.float32)
        wvt[EP * 1 : EP * 1 + E, :] = Wv.T  # predicates = component 1
        m["w1t"] = w1t.astype(bf)
        m["wvt"] = wvt.astype(bf)
        m["w2t"] = np.ascontiguousarray(W2.T).astype(bf)
        m["b1d"] = np.ascontiguousarray(b1.reshape(4, 128).T)
        m["table"] = np.ascontiguousarray(
            np.asarray(inputs["event_table"], np.float32)
        )
        m["ctxidx"] = np.ascontiguousarray(
            ctx.astype(np.int32).transpose(1, 0, 2).reshape(128, BC * C)
        )
        m["evidx"] = ev.astype(np.int32)
    return m


def kernel(**inputs) -> np.ndarray:
    # fast path requires every shard's unique row count to fit int16
    fast = True
    ctx = np.asarray(inputs["batch_context"], np.int64)
    ev = np.asarray(inputs["batch_event"], np.int64)
    for core in range(NCORES):
        sl = slice(core * BC, (core + 1) * BC)
        nuniq = len(np.unique(np.concatenate(
            [ctx[sl].reshape(-1), ev[sl].reshape(-1)])))
        if nuniq > CT:
            fast = False
            break
    nc = _build_program(fast)
    in_maps = [_prep_core_inputs(inputs, core, fast) for core in range(NCORES)]
    res = run_bass_kernel_spmd(nc, in_maps, core_ids=list(range(NCORES)))
    return np.concatenate([r["out"] for r in res.results], axis=0)


if __name__ == "__main__":
    nc = _build_program(True)
    print("program built ok")


# revision 13
# speedup vs baseline: 1.6453x; 1.3335x over previous
"""Trainium2 Bass kernel for nn_EventPairCompositionModel.

Strategy (data-parallel over batch, 8 cores, B=512 -> 64 per core):
  - Host compacts the 60MB f32 table per core to the ~24K unique rows its
    shard touches, stored fp8e4m3 (x16 scale) with rows padded to 512B,
    indices remapped to int16.  SWDGE dma_gather (transpose mode) fetches
    embeddings; its 16-bit transpose granularity lands fp8 element pairs
    as adjacent bytes - exactly the moving-operand layout fp8 DoubleRow
    matmuls consume (2 K-tiles per instruction, 2x PE throughput).
  - MLP1 (1200->512) and MLP2 (512->256) run fp8 DoubleRow; PSUM stays
    fp32.  MLP1's ReLU+bias+requant (x256) is a single vector-engine
    tensor_scalar (scale folded to 1 by the 16*16 weight/act scales);
    MLP2's ReLU descales by 1/4096 on the scalar engine into bf16.
  - Cosine numerators/denominators via small bf16 matmuls landing n-on-
    partitions; norms folded through one exp(-0.5 ln x); KNRM pooling via
    ones-matmul partition reductions; final linear + sigmoid on-chip.
  - If a shard touches >32767 unique rows (can't happen for random
    inputs), falls back to the original bf16 indirect-DMA path.
All 8 cores run the identical program on their own batch shard (SPMD, no
collectives); host concatenates the 8 (64,1) outputs.
"""

import numpy as np
import ml_dtypes

import concourse.bacc as bacc
import concourse.bass as bass
import concourse.tile as tile
import concourse.mybir as mybir
from concourse.bass import IndirectOffsetOnAxis
from concourse.bass_utils import run_bass_kernel_spmd
from concourse import library_config

F32 = mybir.dt.float32
BF16 = mybir.dt.bfloat16
F8 = mybir.dt.float8e4
I16 = mybir.dt.int16
I32 = mybir.dt.int32
AF = mybir.ActivationFunctionType
DR = mybir.MatmulPerfMode.DoubleRow

# Problem shapes (hardcoded per spec)
B, N, C, E = 512, 128, 4, 300
V = 50000
H1, H2 = 512, 256
NF, NK = 8, 11
NCORES = 8
BC = B // NCORES          # 64 batches per core
CT = 32768                # compact table rows (int16-indexable)
GROUPS = (BC * N) // 512  # 16 groups of 512 (b,n) pairs
SUBT = 4                  # batches per group
EB = 128                  # event-path width (64 real b + 64 junk)

# fast path (fp8)
EP8 = 256                 # fp8 elems gathered per row (elems 0..255; the
                          # 44-elem remainder per component ships as a host-
                          # packed linear block)
KT8 = 10                  # MLP1 K-tiles: t=2c+b holds elems e=2p+b of comp c;
                          # t=8+rt holds remainder r=2p+rt -> comp r//64,
                          # elem 256 + r%64 (zero-padded past 44)
SA = 16.0                 # table scale
SW = 16.0                 # W1/W2/Wv scale
C1 = 256.0                # s1 requant scale (= SA*SW so act1 scale == 1)
SE = 16.0                 # s2/eh2 requant scale (fp8 cosine path)

# slow path (bf16)
EP = 384                  # padded embedding stride (768B)
CE = C * EP               # 1536
KT = CE // 128            # 12 K-tiles

MUS = [1.0, 0.9, 0.7, 0.5, 0.3, 0.1, -0.1, -0.3, -0.5, -0.7, -0.9]
SIGMAS = [1e-3] + [0.1] * 10

_PROGRAM_CACHE = {}


def _spread_swdge_queues(nc):
    # Spread SWDGE gathers across the 4 queues. The ucode locks each DMASW
    # semaphore lane to one queue, and Tile assigns lanes round-robin in
    # scheduled order, so derive queue from the assigned lane post-compile.
    import re as _re
    for blk in nc.m.functions[0].blocks:
        for inst in blk.instructions:
            if type(inst).__name__ == "InstDMAGatherAnt":
                for u in inst.sync_info.on_update:
                    m = _re.match(r"DMASW(\d+)_", u.ant_name or "")
                    if m:
                        inst.queue_num = int(m.group(1)) % 4
                        break


def _build_fast():
    nc = bacc.Bacc("TRN2", target_bir_lowering=False, debug=False, num_swdge_queues=4)

    # ---- DRAM I/O ----
    ctab = nc.dram_tensor("ctab", (CT, EP8), F8, kind="ExternalInput")
    cidx = nc.dram_tensor("cidx", (128, GROUPS * C * 32), I16, kind="ExternalInput")
    eidx = nc.dram_tensor("eidx", (128, 32), I16, kind="ExternalInput")
    remd = nc.dram_tensor("remd", (128, GROUPS * 1024), F8, kind="ExternalInput")
    remed = nc.dram_tensor("remed", (128, 256), F8, kind="ExternalInput")
    w1t = nc.dram_tensor("w1t", (KT8 * 128, H1), F8, kind="ExternalInput")
    w2t = nc.dram_tensor("w2t", (H1, H2), F8, kind="ExternalInput")
    wvt = nc.dram_tensor("wvt", (4 * 128, 9), F8, kind="ExternalInput")
    b1d = nc.dram_tensor("b1d", (128, 4), F32, kind="ExternalInput")
    b2d = nc.dram_tensor("b2d", (128, 2), F32, kind="ExternalInput")
    bvd = nc.dram_tensor("bvd", (9, 1), F32, kind="ExternalInput")
    wct = nc.dram_tensor("wct", (128, 1), F32, kind="ExternalInput")
    wckp = nc.dram_tensor("wckp", (1, NK), F32, kind="ExternalInput")
    bcd = nc.dram_tensor("bcd", (1, 1), F32, kind="ExternalInput")
    ndsq = nc.dram_tensor("ndsq", (9, BC), F32, kind="ExternalInput")
    featT = nc.dram_tensor("featT", (NF, BC), F32, kind="ExternalInput")
    out_d = nc.dram_tensor("out", (BC, 1), F32, kind="ExternalOutput")

    with tile.TileContext(nc) as tc:
        with (
            tc.tile_pool(name="consts", bufs=1) as cpool,
            tc.tile_pool(name="xt", bufs=4) as xtpool,
            tc.tile_pool(name="rem", bufs=4) as rempool,
            tc.tile_pool(name="s1", bufs=3) as s1pool,
            tc.tile_pool(name="s2", bufs=3) as s2pool,
            tc.tile_pool(name="csq", bufs=3) as csqpool,
            tc.tile_pool(name="small", bufs=2) as smpool,
            tc.tile_pool(name="pm1", bufs=2, space="PSUM") as pm1,
            tc.tile_pool(name="pm2", bufs=2, space="PSUM") as pm2,
            tc.tile_pool(name="ptn", bufs=1, space="PSUM") as ptn,
            tc.tile_pool(name="pmisc", bufs=2, space="PSUM") as pmisc,
        ):
            # ---- load constants ----
            nc.gpsimd.load_library(library_config.mlp)
            cidx_s = cpool.tile([128, GROUPS * C * 32], I16)
            nc.sync.dma_start(cidx_s[:], cidx.ap())
            eidx_s = cpool.tile([128, 32], I16)
            nc.sync.dma_start(eidx_s[:], eidx.ap())
            w1t_s = cpool.tile([128, KT8 * H1], F8)
            nc.sync.dma_start(
                w1t_s[:].rearrange("p (t m) -> p t m", t=KT8),
                w1t.ap().rearrange("(t p) m -> p t m", p=128),
            )
            w2t_s = cpool.tile([128, 4 * H2], F8)
            nc.scalar.dma_start(
                w2t_s[:].rearrange("p (t m) -> p t m", t=4),
                w2t.ap().rearrange("(t p) m -> p t m", p=128),
            )
            wvt_s = cpool.tile([128, 4 * 9], F8)
            nc.scalar.dma_start(
                wvt_s[:].rearrange("p (t m) -> p t m", t=4),
                wvt.ap().rearrange("(t p) m -> p t m", p=128),
            )
            b1_s = cpool.tile([128, 4], F32)
            nc.sync.dma_start(b1_s[:], b1d.ap())
            b2_s = cpool.tile([128, 2], F32)
            nc.sync.dma_start(b2_s[:], b2d.ap())
            bv_s = cpool.tile([9, 1], F32)
            nc.sync.dma_start(bv_s[:], bvd.ap())
            wct_s = cpool.tile([128, 1], F32)
            nc.sync.dma_start(wct_s[:], wct.ap())
            wckp_s = cpool.tile([1, NK], F32)
            nc.sync.dma_start(wckp_s[:], wckp.ap())
            bc_s = cpool.tile([1, 1], F32)
            nc.sync.dma_start(bc_s[:], bcd.ap())
            ndsq_s = cpool.tile([9, BC], F32)
            nc.sync.dma_start(ndsq_s[:], ndsq.ap())
            feat_s = cpool.tile([128, BC], F32)
            nc.vector.memset(feat_s[:], 0.0)
            nc.sync.dma_start(feat_s[64 : 64 + NF, :], featT.ap())
            onesrow_s = cpool.tile([1, 128], F32)
            nc.vector.memset(onesrow_s[:], 1.0)
            onesf_s = cpool.tile([128, 1], F32)
            nc.vector.memset(onesf_s[:], 1.0)
            ones8_s = cpool.tile([128, 2], F8)
            nc.vector.memset(ones8_s[:], 1.0)
            eps_s = cpool.tile([128, 1], F32)
            nc.vector.memset(eps_s[:], 1e-20)
            lnse_s = cpool.tile([128, 1], F32)
            nc.vector.memset(lnse_s[:], -float(np.log(SE)))
            mub_s = cpool.tile([128, NK], F32)
            for k in range(NK):
                nc.vector.memset(mub_s[:, k : k + 1], -MUS[k])

            remed_s = cpool.tile([128, 256], F8)
            nc.sync.dma_start(remed_s[:], remed.ap())

            # views: pair u = t//2, "two" = the byte dim consumed by DoubleRow
            w1_v = w1t_s[:].rearrange("p (u two m) -> p u two m", u=KT8 // 2, two=2)
            w2_v = w2t_s[:].rearrange("p (q two m) -> p q two m", q=2, two=2)
            ones8_v = ones8_s[:].rearrange("p (two i) -> p two i", two=2)
            remed_v = remed_s[:].rearrange("p (two i) -> p two i", two=2)

            # ---- event path (EB=128 lanes, only 0..63 meaningful) ----
            xeT = cpool.tile([128, (EP8 // 128) * 512], F8)  # 1024/partition
            nc.gpsimd.dma_gather(
                out_ap=xeT[:].rearrange("p (j i) -> p j i", j=EP8 // 128),
                in_ap=ctab.ap(),
                idxs_ap=eidx_s[:],
                num_idxs=512,
                num_idxs_reg=512,
                elem_size=EP8,
                transpose=True,
            )
            # flat = 256*c + 2*eb + b; elem = 2p + b of component c
            xeT_v = xeT[:].rearrange("p (c i b) -> p c b i", c=C, b=2)

            s1e = cpool.tile([128, 4 * EB], F8)
            for m in range(4):
                pe = pmisc.tile([128, EB], F32, tag="pmisc", name="pe")
                for c in range(C):
                    nc.tensor.matmul(
                        pe[:],
                        w1_v[:, c, :, 128 * m : 128 * m + 128],
                        xeT_v[:, c],
                        start=(c == 0),
                        stop=False,
                        perf_mode=DR,
                    )
                nc.tensor.matmul(
                    pe[:], w1_v[:, 4, :, 128 * m : 128 * m + 128], remed_v,
                    start=False, stop=True, perf_mode=DR,
                )
                nc.vector.tensor_scalar(
                    s1e[:, EB * m : EB * (m + 1)], pe[:],
                    b1_s[:, m : m + 1], 0.0,
                    mybir.AluOpType.add, mybir.AluOpType.max,
                )

            s1e_v = s1e[:].rearrange("p (q two i) -> p q two i", q=2, two=2)
            eh2q = cpool.tile([128, 2 * EB], F8)
            for mm in range(2):
                pe2 = pmisc.tile([128, EB], F32, tag="pmisc", name="pe2")
                for q in range(2):
                    nc.tensor.matmul(
                        pe2[:],
                        w2_v[:, q, :, 128 * mm : 128 * mm + 128],
                        s1e_v[:, q],
                        start=(q == 0),
                        stop=(q == 1),
                        perf_mode=DR,
                    )
                nc.scalar.activation(
                    eh2q[:, EB * mm : EB * (mm + 1)], pe2[:], AF.Relu,
                    bias=b2_s[:, mm : mm + 1], scale=SE / (SW * C1),
                )
            eh2q_v = eh2q[:].rearrange("p (two i) -> p two i", two=2)

            # variances -> dist_emb rows 32..40 of feat_s (regular fp8
            # matmuls: DoubleRow rejects 9-wide stationaries)
            pv = pmisc.tile([9, EB], F32, tag="pmisc", name="pv")
            for bb in range(2):
                nc.tensor.matmul(
                    pv[:], wvt_s[:, 9 * bb : 9 * (bb + 1)],
                    xeT_v[:, 1, bb, :],
                    start=(bb == 0), stop=False,
                )
            for rt in range(2):
                nc.tensor.matmul(
                    pv[:], wvt_s[:, 9 * (2 + rt) : 9 * (3 + rt)],
                    remed_s[:, 128 * rt : 128 * (rt + 1)],
                    start=False, stop=(rt == 1),
                )
            ez_s = smpool.tile([9, EB], F32)
            nc.scalar.activation(
                ez_s[:], pv[:], AF.Exp, bias=bv_s[:], scale=1.0 / (SA * SW)
            )
            ez1_s = smpool.tile([9, EB], F32)
            nc.vector.tensor_scalar_add(ez1_s[:], ez_s[:], 1.0)
            var_s = smpool.tile([9, EB], F32)
            nc.scalar.activation(var_s[:], ez1_s[:], AF.Ln)
            rv_s = smpool.tile([9, EB], F32)
            nc.vector.reciprocal(rv_s[:], var_s[:])
            q_s = smpool.tile([9, BC], F32)
            nc.vector.tensor_mul(q_s[:], ndsq_s[:], rv_s[:, 0:BC])
            nc.scalar.activation(feat_s[32:41, :], q_s[:], AF.Exp)

            # |e|^2 per b (x SE^2/SE = 16), broadcast to all 128 partitions
            esq_q = smpool.tile([128, 2 * EB], F8, tag="esq")
            nc.vector.scalar_tensor_tensor(
                esq_q[:], eh2q[:], 1.0 / SE, eh2q[:],
                mybir.AluOpType.mult, mybir.AluOpType.mult,
            )
            pne = pmisc.tile([1, EB], F32, tag="pmisc", name="pne")
            for k in range(2):
                nc.tensor.matmul(
                    pne[:], ones8_s[:, 0:1], esq_q[:, EB * k : EB * (k + 1)],
                    start=(k == 0), stop=(k == 1),
                )
            ne2_s = smpool.tile([1, BC], F32)
            nc.scalar.copy(ne2_s[:], pne[:, 0:BC])
            pne2bc = pmisc.tile([128, BC], F32, tag="pmisc", name="pne2bc")
            nc.tensor.matmul(
                pne2bc[:], onesrow_s[:], ne2_s[:], start=True, stop=True
            )
            ne2bc_s = cpool.tile([128, BC], F32)
            nc.scalar.copy(ne2bc_s[:], pne2bc[:])

            # persistent SBUF accumulators, n on partitions, b on free
            traw_s = cpool.tile([128, BC], F32)
            ncsq_s = cpool.tile([128, BC], F32)

            # ---- context groups ----
            for g in range(GROUPS):
                xt = xtpool.tile([128, C * 1024], F8)
                for c in range(C):
                    nc.gpsimd.dma_gather(
                        out_ap=xt[:, 1024 * c : 1024 * (c + 1)].rearrange(
                            "p (j i) -> p j i", j=EP8 // 128
                        ),
                        in_ap=ctab.ap(),
                        idxs_ap=cidx_s[:, 32 * (C * g + c) : 32 * (C * g + c + 1)],
                        num_idxs=512,
                        num_idxs_reg=512,
                        elem_size=EP8,
                        transpose=True,
                    )
                rem_s = rempool.tile([128, 1024], F8, tag="rem", name="rem")
                nc.sync.dma_start(rem_s[:], remd.ap()[:, 1024 * g : 1024 * (g + 1)])
                # flat = 1024c + 2i + b, i = z*128 + n
                xt_v = xt[:].rearrange("p (c i b) -> p c b i", c=C, b=2)
                rem_v = rem_s[:].rearrange("p (two i) -> p two i", two=2)

                s1 = s1pool.tile([128, 4 * 512], F8, tag="s1", name="s1")
                for m in range(4):
                    p1 = pm1.tile([128, 512], F32)
                    for c in range(C):
                        nc.tensor.matmul(
                            p1[:],
                            w1_v[:, c, :, 128 * m : 128 * m + 128],
                            xt_v[:, c],
                            start=(c == 0),
                            stop=False,
                            perf_mode=DR,
                        )
                    nc.tensor.matmul(
                        p1[:], w1_v[:, 4, :, 128 * m : 128 * m + 128], rem_v,
                        start=False, stop=True, perf_mode=DR,
                    )
                    if m < 2:
                        nc.scalar.activation(
                            s1[:, 512 * m : 512 * (m + 1)], p1[:], AF.Relu,
                            bias=b1_s[:, m : m + 1],
                        )
                    else:
                        nc.vector.tensor_scalar(
                            s1[:, 512 * m : 512 * (m + 1)], p1[:],
                            b1_s[:, m : m + 1], 0.0,
                            mybir.AluOpType.add, mybir.AluOpType.max,
                        )

                s1_v = s1[:].rearrange("p (q two i) -> p q two i", q=2, two=2)
                s2q = s2pool.tile([128, 2 * 512], F8, tag="s2q", name="s2q")
                for mm in range(2):
                    p2 = pm2.tile([128, 512], F32)
                    for q in range(2):
                        nc.tensor.matmul(
                            p2[:],
                            w2_v[:, q, :, 128 * mm : 128 * mm + 128],
                            s1_v[:, q],
                            start=(q == 0),
                            stop=(q == 1),
                            perf_mode=DR,
                        )
                    nc.scalar.activation(
                        s2q[:, 512 * mm : 512 * (mm + 1)], p2[:], AF.Relu,
                        bias=b2_s[:, mm : mm + 1], scale=SE / (SW * C1),
                    )
                s2q_v = s2q[:].rearrange("p (two i) -> p two i", two=2)

                csq_q = csqpool.tile([128, 2 * 512], F8, tag="csq", name="csq")
                nc.vector.scalar_tensor_tensor(
                    csq_q[:], s2q[:], 1.0 / SE, s2q[:],
                    mybir.AluOpType.mult, mybir.AluOpType.mult,
                )
                csq_v = csq_q[:].rearrange("p (two i) -> p two i", two=2)

                # raw dots and |c|^2, n on partitions, one column per b
                pT = ptn.tile([128, SUBT], F32, tag="pT", name="pT")
                pN = ptn.tile([128, SUBT], F32, tag="pN", name="pN")
                for s in range(SUBT):
                    b = SUBT * g + s
                    nc.tensor.matmul(
                        pT[:, s : s + 1],
                        s2q_v[:, :, 128 * s : 128 * (s + 1)],
                        eh2q_v[:, :, b : b + 1],
                        start=True, stop=True, perf_mode=DR,
                    )
                    nc.tensor.matmul(
                        pN[:, s : s + 1],
                        csq_v[:, :, 128 * s : 128 * (s + 1)],
                        ones8_v,
                        start=True, stop=True, perf_mode=DR,
                    )
                nc.scalar.copy(traw_s[:, SUBT * g : SUBT * (g + 1)], pT[:])
                nc.scalar.copy(ncsq_s[:, SUBT * g : SUBT * (g + 1)], pN[:])

            # ---- kernel pooling (tiles are [n=128, b=64]) ----
            # traw = SE^2*raw, prodn = SE^2*|c|^2|e|^2: fold the extra SE^2
            # out via the exp bias (-ln SE^2 * 0.5 ... = -ln(SE))
            prodn = smpool.tile([128, BC], F32, tag="prodn")
            nc.vector.tensor_mul(prodn[:], ncsq_s[:], ne2bc_s[:])
            lnp = smpool.tile([128, BC], F32, tag="lnp")
            nc.scalar.activation(lnp[:], prodn[:], AF.Ln, bias=eps_s[:])
            nrmf = smpool.tile([128, BC], F32, tag="nrmf")
            nc.scalar.activation(
                nrmf[:], lnp[:], AF.Exp, scale=-0.5, bias=lnse_s[:]
            )
            trans = cpool.tile([128, BC], F32)
            nc.vector.tensor_mul(trans[:], traw_s[:], nrmf[:])

            kpp_s = cpool.tile([1, NK * BC], F32)
            for k in range(NK):
                sq = smpool.tile([128, BC], F32, tag="sq", name="sq")
                nc.scalar.activation(
                    sq[:], trans[:], AF.Square, bias=mub_s[:, k : k + 1]
                )
                arg = smpool.tile([128, BC], F32, tag="arg", name="arg")
                nc.vector.tensor_scalar(
                    arg[:], sq[:],
                    -1.0 / (2.0 * SIGMAS[k] ** 2), -87.0,
                    mybir.AluOpType.mult, mybir.AluOpType.max,
                )
                ek = smpool.tile([128, BC], F32, tag="ek", name="ek")
                nc.scalar.activation(ek[:], arg[:], AF.Exp)
                pp = pmisc.tile([1, BC], F32, tag="pmisc", name="pp")
                nc.tensor.matmul(pp[:], onesf_s[:], ek[:], start=True, stop=True)
                nc.scalar.copy(kpp_s[:, BC * k : BC * (k + 1)], pp[:])

            kpc_s = smpool.tile([1, NK * BC], F32, tag="kpc")
            nc.vector.tensor_scalar_max(kpc_s[:], kpp_s[:], 1e-10)
            kpl_s = smpool.tile([1, NK * BC], F32, tag="kpl")
            nc.scalar.activation(kpl_s[:], kpc_s[:], AF.Ln)

            # weighted sum over k: kps[b] = sum_k wckp[k] * kpl[k, b]
            kpw_s = smpool.tile([1, BC * NK], F32, tag="kpw")
            kpl_v = kpl_s[:].rearrange("p (k b) -> p b k", k=NK)
            wck_v = wckp_s[:][:, None, :].broadcast_to([1, BC, NK])
            kpw_v = kpw_s[:].rearrange("p (b k) -> p b k", b=BC)
            nc.vector.tensor_tensor(
                out=kpw_v, in0=kpl_v, in1=wck_v, op=mybir.AluOpType.mult
            )
            kps_s = smpool.tile([1, BC], F32, tag="kps")
            nc.vector.reduce_sum(
                out=kps_s[:], in_=kpw_v, axis=mybir.AxisListType.X
            )

            # ---- final score ----
            psc = pmisc.tile([1, BC], F32, tag="pmisc", name="psc")
            nc.tensor.matmul(psc[:], wct_s[:], feat_s[:], start=True, stop=True)
            tot_s = smpool.tile([1, BC], F32, tag="tot")
            nc.vector.tensor_add(tot_s[:], psc[:], kps_s[:])
            emx = smpool.tile([1, BC], F32, tag="emx")
            nc.scalar.activation(emx[:], tot_s[:], AF.Exp, bias=bc_s[:], scale=-1.0)
            emx1 = smpool.tile([1, BC], F32, tag="emx1")
            nc.vector.tensor_scalar_add(emx1[:], emx[:], 1.0)
            outs = smpool.tile([1, BC], F32, tag="outs")
            nc.vector.reciprocal(outs[:], emx1[:])
            nc.sync.dma_start(out_d.ap().rearrange("b one -> one b"), outs[:])

    nc.compile()
    _spread_swdge_queues(nc)
    return nc


def _build_slow():
    """Original bf16 fallback: indirect-DMA gather of the full f32 table."""
    nc = bacc.Bacc("TRN2", target_bir_lowering=False, debug=False, num_swdge_queues=4)

    ctab = nc.dram_tensor("table", (V + 1, E), F32, kind="ExternalInput")
    cidx = nc.dram_tensor("ctxidx", (128, BC * C), I32, kind="ExternalInput")
    eidx = nc.dram_tensor("evidx", (BC, C), I32, kind="ExternalInput")
    w1t = nc.dram_tensor("w1t", (CE, H1), BF16, kind="ExternalInput")
    w2t = nc.dram_tensor("w2t", (H1, H2), BF16, kind="ExternalInput")
    wvt = nc.dram_tensor("wvt", (CE, 9), BF16, kind="ExternalInput")
    b1d = nc.dram_tensor("b1d", (128, 4), F32, kind="ExternalInput")
    b2d = nc.dram_tensor("b2d", (128, 2), F32, kind="ExternalInput")
    bvd = nc.dram_tensor("bvd", (9, 1), F32, kind="ExternalInput")
    wct = nc.dram_tensor("wct", (128, 1), F32, kind="ExternalInput")
    wckp = nc.dram_tensor("wckp", (1, NK), F32, kind="ExternalInput")
    bcd = nc.dram_tensor("bcd", (1, 1), F32, kind="ExternalInput")
    ndsq = nc.dram_tensor("ndsq", (9, BC), F32, kind="ExternalInput")
    featT = nc.dram_tensor("featT", (NF, BC), F32, kind="ExternalInput")
    out_d = nc.dram_tensor("out", (BC, 1), F32, kind="ExternalOutput")

    with tile.TileContext(nc) as tc:
        with (
            tc.tile_pool(name="consts", bufs=1) as cpool,
            tc.tile_pool(name="xg", bufs=4) as xgpool,
            tc.tile_pool(name="xt", bufs=4) as xtpool,
            tc.tile_pool(name="s1", bufs=8) as s1pool,
            tc.tile_pool(name="s2", bufs=4) as s2pool,
            tc.tile_pool(name="csq", bufs=4) as csqpool,
            tc.tile_pool(name="small", bufs=2) as smpool,
            tc.tile_pool(name="pm1", bufs=2, space="PSUM") as pm1,
            tc.tile_pool(name="pm2", bufs=2, space="PSUM") as pm2,
            tc.tile_pool(name="ptn", bufs=1, space="PSUM") as ptn,
            tc.tile_pool(name="pmisc", bufs=2, space="PSUM") as pmisc,
        ):
            w1t_s = cpool.tile([128, KT * H1], BF16)
            nc.sync.dma_start(
                w1t_s[:].rearrange("p (t m) -> p t m", t=KT),
                w1t.ap().rearrange("(t p) m -> p t m", p=128),
            )
            w2t_s = cpool.tile([128, 4 * H2], BF16)
            nc.scalar.dma_start(
                w2t_s[:].rearrange("p (t m) -> p t m", t=4),
                w2t.ap().rearrange("(t p) m -> p t m", p=128),
            )
            wvt_s = cpool.tile([128, KT * 9], BF16)
            nc.scalar.dma_start(
                wvt_s[:].rearrange("p (t m) -> p t m", t=KT),
                wvt.ap().rearrange("(t p) m -> p t m", p=128),
            )
            b1_s = cpool.tile([128, 4], F32)
            nc.sync.dma_start(b1_s[:], b1d.ap())
            b2_s = cpool.tile([128, 2], F32)
            nc.sync.dma_start(b2_s[:], b2d.ap())
            bv_s = cpool.tile([9, 1], F32)
            nc.sync.dma_start(bv_s[:], bvd.ap())
            wct_s = cpool.tile([128, 1], F32)
            nc.sync.dma_start(wct_s[:], wct.ap())
            wckp_s = cpool.tile([1, NK], F32)
            nc.sync.dma_start(wckp_s[:], wckp.ap())
            bc_s = cpool.tile([1, 1], F32)
            nc.sync.dma_start(bc_s[:], bcd.ap())
            cidx_s = cpool.tile([128, BC * C], I32)
            nc.sync.dma_start(cidx_s[:], cidx.ap())
            eidx_s = cpool.tile([BC, C], I32)
            nc.sync.dma_start(eidx_s[:], eidx.ap())
            ndsq_s = cpool.tile([9, BC], F32)
            nc.sync.dma_start(ndsq_s[:], ndsq.ap())
            feat_s = cpool.tile([128, BC], F32)
            nc.vector.memset(feat_s[:], 0.0)
            nc.sync.dma_start(feat_s[64 : 64 + NF, :], featT.ap())
            ones_s = cpool.tile([128, 1], BF16)
            nc.vector.memset(ones_s[:], 1.0)
            onesrow_s = cpool.tile([1, 128], F32)
            nc.vector.memset(onesrow_s[:], 1.0)
            onesf_s = cpool.tile([128, 1], F32)
            nc.vector.memset(onesf_s[:], 1.0)
            eps_s = cpool.tile([128, 1], F32)
            nc.vector.memset(eps_s[:], 1e-20)
            mub_s = cpool.tile([128, NK], F32)
            for k in range(NK):
                nc.vector.memset(mub_s[:, k : k + 1], -MUS[k])

            # ---- event path ----
            xeT = cpool.tile([128, KT * EB], BF16)
            xe = cpool.tile([EB, CE], BF16)
            nc.vector.memset(xe[:], 0.0)
            nc.gpsimd.indirect_dma_start(
                out=xe[0:BC, :].rearrange("p (c e) -> p c e", c=C)[:, :, 0:E],
                out_offset=None,
                in_=ctab.ap(),
                in_offset=IndirectOffsetOnAxis(ap=eidx_s[:], axis=0),
            )
            nc.sync.dma_start_transpose(
                xeT[:].rearrange("p (j i) -> p j i", j=KT), xe[:]
            )

            def xeT_k(j):
                return xeT[:, EB * j : EB * (j + 1)]

            s1e = cpool.tile([128, 4 * EB], BF16)
            for m in range(4):
                pe = pmisc.tile([128, EB], F32, tag="pmisc", name="pe")
                for j in range(KT):
                    nc.tensor.matmul(
                        pe[:],
                        w1t_s[:, H1 * j + 128 * m : H1 * j + 128 * m + 128],
                        xeT_k(j),
                        start=(j == 0),
                        stop=(j == KT - 1),
                    )
                nc.scalar.activation(
                    s1e[:, EB * m : EB * (m + 1)], pe[:], AF.Relu,
                    bias=b1_s[:, m : m + 1],
                )

            eh2 = [
                cpool.tile([128, EB], BF16, tag=f"eh2_{k}", name=f"eh2_{k}")
                for k in range(2)
            ]
            for m in range(2):
                pe2 = pmisc.tile([128, EB], F32, tag="pmisc", name="pe2")
                for j in range(4):
                    nc.tensor.matmul(
                        pe2[:],
                        w2t_s[:, H2 * j + 128 * m : H2 * j + 128 * m + 128],
                        s1e[:, EB * j : EB * (j + 1)],
                        start=(j == 0),
                        stop=(j == 3),
                    )
                nc.scalar.activation(
                    eh2[m][:], pe2[:], AF.Relu, bias=b2_s[:, m : m + 1]
                )

            pv = pmisc.tile([9, EB], F32, tag="pmisc", name="pv")
            for j in range(KT):
                nc.tensor.matmul(
                    pv[:],
                    wvt_s[:, 9 * j : 9 * (j + 1)],
                    xeT_k(j),
                    start=(j == 0),
                    stop=(j == KT - 1),
                )
            ez_s = smpool.tile([9, EB], F32)
            nc.scalar.activation(ez_s[:], pv[:], AF.Exp, bias=bv_s[:])
            ez1_s = smpool.tile([9, EB], F32)
            nc.vector.tensor_scalar_add(ez1_s[:], ez_s[:], 1.0)
            var_s = smpool.tile([9, EB], F32)
            nc.scalar.activation(var_s[:], ez1_s[:], AF.Ln)
            rv_s = smpool.tile([9, EB], F32)
            nc.vector.reciprocal(rv_s[:], var_s[:])
            q_s = smpool.tile([9, BC], F32)
            nc.vector.tensor_mul(q_s[:], ndsq_s[:], rv_s[:, 0:BC])
            nc.scalar.activation(feat_s[32:41, :], q_s[:], AF.Exp)

            esq = [
                smpool.tile([128, EB], BF16, tag=f"esq_{k}", name=f"esq_{k}")
                for k in range(2)
            ]
            for k in range(2):
                nc.vector.tensor_mul(esq[k][:], eh2[k][:], eh2[k][:])
            pne = pmisc.tile([1, EB], F32, tag="pmisc", name="pne")
            for k in range(2):
                nc.tensor.matmul(
                    pne[:], ones_s[:], esq[k][:], start=(k == 0), stop=(k == 1)
                )
            ne2_s = smpool.tile([1, BC], F32)
            nc.scalar.copy(ne2_s[:], pne[:, 0:BC])
            pne2bc = pmisc.tile([128, BC], F32, tag="pmisc", name="pne2bc")
            nc.tensor.matmul(
                pne2bc[:], onesrow_s[:], ne2_s[:], start=True, stop=True
            )
            ne2bc_s = cpool.tile([128, BC], F32)
            nc.scalar.copy(ne2bc_s[:], pne2bc[:])

            traw_s = cpool.tile([128, BC], F32)
            ncsq_s = cpool.tile([128, BC], F32)

            for g in range(GROUPS):
                xt = xtpool.tile([128, KT * 512], BF16)
                xg = xgpool.tile([128, SUBT * CE], BF16)
                nc.vector.memset(
                    xg[:].rearrange("p (q e) -> p q e", e=EP)[:, :, E:EP],
                    0.0,
                )
                for s in range(SUBT):
                    nc.gpsimd.indirect_dma_start(
                        out=xg[:]
                        .rearrange("p (q c e) -> p q c e", q=SUBT, c=C)[
                            :, s, :, 0:E
                        ],
                        out_offset=None,
                        in_=ctab.ap(),
                        in_offset=IndirectOffsetOnAxis(
                            ap=cidx_s[
                                :, (SUBT * g + s) * C : (SUBT * g + s + 1) * C
                            ],
                            axis=0,
                        ),
                    )
                for s in range(SUBT):
                    nc.sync.dma_start_transpose(
                        xt[:].rearrange(
                            "p (j z i) -> p j z i", j=KT, z=SUBT
                        )[:, :, s, :],
                        xg[:, CE * s : CE * (s + 1)],
                    )

                def xt_k(j):
                    return xt[:, 512 * j : 512 * (j + 1)]

                s1 = [
                    s1pool.tile([128, 512], BF16, tag=f"s1_{m}", name=f"s1_{m}")
                    for m in range(4)
                ]
                for m in range(4):
                    p1 = pm1.tile([128, 512], F32)
                    for j in range(KT):
                        nc.tensor.matmul(
                            p1[:],
                            w1t_s[:, H1 * j + 128 * m : H1 * j + 128 * m + 128],
                            xt_k(j),
                            start=(j == 0),
                            stop=(j == KT - 1),
                        )
                    nc.scalar.activation(
                        s1[m][:], p1[:], AF.Relu, bias=b1_s[:, m : m + 1]
                    )

                s2 = [
                    s2pool.tile([128, 512], BF16, tag=f"s2_{m}", name=f"s2_{m}")
                    for m in range(2)
                ]
                for m in range(2):
                    p2 = pm2.tile([128, 512], F32)
                    for j in range(4):
                        nc.tensor.matmul(
                            p2[:],
                            w2t_s[:, H2 * j + 128 * m : H2 * j + 128 * m + 128],
                            s1[j][:],
                            start=(j == 0),
                            stop=(j == 3),
                        )
                    nc.scalar.activation(
                        s2[m][:], p2[:], AF.Relu, bias=b2_s[:, m : m + 1]
                    )

                csq = [
                    csqpool.tile([128, 512], BF16, tag=f"csq_{m}", name=f"csq_{m}")
                    for m in range(2)
                ]
                for m in range(2):
                    nc.vector.tensor_mul(csq[m][:], s2[m][:], s2[m][:])

                pT = ptn.tile([128, SUBT], F32, tag="pT", name="pT")
                pN = ptn.tile([128, SUBT], F32, tag="pN", name="pN")
                for s in range(SUBT):
                    b = SUBT * g + s
                    for k in range(2):
                        nc.tensor.matmul(
                            pT[:, s : s + 1],
                            s2[k][:, 128 * s : 128 * (s + 1)],
                            eh2[k][:, b : b + 1],
                            start=(k == 0),
                            stop=(k == 1),
                        )
                    for k in range(2):
                        nc.tensor.matmul(
                            pN[:, s : s + 1],
                            csq[k][:, 128 * s : 128 * (s + 1)],
                            ones_s[:],
                            start=(k == 0),
                            stop=(k == 1),
                        )
                nc.scalar.copy(traw_s[:, SUBT * g : SUBT * (g + 1)], pT[:])
                nc.scalar.copy(ncsq_s[:, SUBT * g : SUBT * (g + 1)], pN[:])

            prodn = smpool.tile([128, BC], F32, tag="prodn")
            nc.vector.tensor_mul(prodn[:], ncsq_s[:], ne2bc_s[:])
            lnp = smpool.tile([128, BC], F32, tag="lnp")
            nc.scalar.activation(lnp[:], prodn[:], AF.Ln, bias=eps_s[:])
            nrmf = smpool.tile([128, BC], F32, tag="nrmf")
            nc.scalar.activation(nrmf[:], lnp[:], AF.Exp, scale=-0.5)
            trans = cpool.tile([128, BC], F32)
            nc.vector.tensor_mul(trans[:], traw_s[:], nrmf[:])

            kpp_s = cpool.tile([1, NK * BC], F32)
            for k in range(NK):
                sq = smpool.tile([128, BC], F32, tag="sq", name="sq")
                nc.scalar.activation(
                    sq[:], trans[:], AF.Square, bias=mub_s[:, k : k + 1]
                )
                arg = smpool.tile([128, BC], F32, tag="arg", name="arg")
                nc.vector.tensor_scalar(
                    arg[:], sq[:],
                    -1.0 / (2.0 * SIGMAS[k] ** 2), -87.0,
                    mybir.AluOpType.mult, mybir.AluOpType.max,
                )
                ek = smpool.tile([128, BC], F32, tag="ek", name="ek")
                nc.scalar.activation(ek[:], arg[:], AF.Exp)
                pp = pmisc.tile([1, BC], F32, tag="pmisc", name="pp")
                nc.tensor.matmul(pp[:], onesf_s[:], ek[:], start=True, stop=True)
                nc.scalar.copy(kpp_s[:, BC * k : BC * (k + 1)], pp[:])

            kpc_s = smpool.tile([1, NK * BC], F32, tag="kpc")
            nc.vector.tensor_scalar_max(kpc_s[:], kpp_s[:], 1e-10)
            kpl_s = smpool.tile([1, NK * BC], F32, tag="kpl")
            nc.scalar.activation(kpl_s[:], kpc_s[:], AF.Ln)

            kpw_s = smpool.tile([1, BC * NK], F32, tag="kpw")
            kpl_v = kpl_s[:].rearrange("p (k b) -> p b k", k=NK)
            wck_v = wckp_s[:][:, None, :].broadcast_to([1, BC, NK])
            kpw_v = kpw_s[:].rearrange("p (b k) -> p b k", b=BC)
            nc.vector.tensor_tensor(
                out=kpw_v, in0=kpl_v, in1=wck_v, op=mybir.AluOpType.mult
            )
            kps_s = smpool.tile([1, BC], F32, tag="kps")
            nc.vector.reduce_sum(
                out=kps_s[:], in_=kpw_v, axis=mybir.AxisListType.X
            )

            psc = pmisc.tile([1, BC], F32, tag="pmisc", name="psc")
            nc.tensor.matmul(psc[:], wct_s[:], feat_s[:], start=True, stop=True)
            tot_s = smpool.tile([1, BC], F32, tag="tot")
            nc.vector.tensor_add(tot_s[:], psc[:], kps_s[:])
            emx = smpool.tile([1, BC], F32, tag="emx")
            nc.scalar.activation(emx[:], tot_s[:], AF.Exp, bias=bc_s[:], scale=-1.0)
            emx1 = smpool.tile([1, BC], F32, tag="emx1")
            nc.vector.tensor_scalar_add(emx1[:], emx[:], 1.0)
            outs = smpool.tile([1, BC], F32, tag="outs")
            nc.vector.reciprocal(outs[:], emx1[:])
            nc.sync.dma_start(out_d.ap().rearrange("b one -> one b"), outs[:])

    nc.compile()
    return nc


def _build_program(fast: bool):
    if fast in _PROGRAM_CACHE:
        return _PROGRAM_CACHE[fast]
    nc = _build_fast() if fast else _build_slow()
    _PROGRAM_CACHE[fast] = nc
    return nc


def _wrap16(flat_idx):
    """int16 index list -> (128, n/16) tile layout: unwrapped[i] =
    tile[i % 16, i // 16], replicated into all 8 16-partition stripes."""
    n = flat_idx.shape[0]
    t = np.zeros((16, n // 16), np.int16)
    t[np.arange(n) % 16, np.arange(n) // 16] = flat_idx
    return np.tile(t, (8, 1))


def _prep_core_inputs(inputs, core, fast):
    """Host-side shard + weight re-layouts for one core."""
    W1 = np.asarray(inputs["W1"], np.float32)
    W2 = np.asarray(inputs["W2"], np.float32)
    Wv = np.asarray(inputs["Wv"], np.float32)
    Wc = np.asarray(inputs["Wc"], np.float32)
    b1 = np.asarray(inputs["b1"], np.float32)
    b2 = np.asarray(inputs["b2"], np.float32)
    bv = np.asarray(inputs["bv"], np.float32)
    bc = np.asarray(inputs["bc"], np.float32)

    sl = slice(core * BC, (core + 1) * BC)
    ev = np.asarray(inputs["batch_event"][sl], np.int64)          # (BC, C)
    feats = np.asarray(inputs["batch_features"][sl], np.float32)  # (BC, NF)
    dists = np.asarray(inputs["batch_distances"][sl], np.float32) # (BC, 9)
    ctx = np.asarray(inputs["batch_context"][sl], np.int64)       # (BC, N, C)

    wc_full = np.zeros((128,), np.float32)
    wc_full[32 : 32 + 9] = Wc[0, 0:9]          # dist_emb block
    wc_full[64 : 64 + NF] = Wc[0, 9 : 9 + NF]  # batch_features block
    wckp = (Wc[0, NF + 9 :] * 0.01).astype(np.float32)  # kp block, 0.01 folded

    m = {
        "bvd": bv.reshape(9, 1),
        "wct": wc_full.reshape(-1, 1),
        "wckp": wckp.reshape(1, NK),
        "bcd": -bc.reshape(1, 1),
        "ndsq": np.ascontiguousarray(-(dists * dists).T),
        "featT": np.ascontiguousarray(feats.T),
        "b2d": np.ascontiguousarray(b2.reshape(2, 128).T),
    }

    if fast:
        f8 = ml_dtypes.float8_e4m3
        p = np.arange(128)
        # W1.T in the transpose-gather's K-permutation: K-tile t = 2c+b holds
        # elements e = 2p+b (<256) of component c; tiles 8+rt hold the
        # remainder r = 2p+rt -> component r//64, element 256 + r%64
        w1t = np.zeros((KT8 * 128, H1), np.float32)
        wvt = np.zeros((4 * 128, 9), np.float32)
        for b in range(2):
            e = 2 * p + b
            for c in range(C):
                w1t[128 * (2 * c + b) + p, :] = W1[:, E * c + e].T
            wvt[128 * b + p, :] = Wv[:, e].T
        for rt in range(2):
            r = 2 * p + rt
            rc, rr = r // 64, r % 64
            msk = rr < E - 256
            w1t[128 * (8 + rt) + p[msk], :] = W1[:, E * rc[msk] + 256 + rr[msk]].T
            mskv = msk & (rc == 1)
            wvt[128 * (2 + rt) + p[mskv], :] = Wv[:, 256 + rr[mskv]].T
        m["w1t"] = (w1t * SW).astype(f8)
        m["wvt"] = (wvt * SW).astype(f8)
        m["w2t"] = (np.ascontiguousarray(W2.T) * SW).astype(f8)
        m["b1d"] = np.ascontiguousarray((b1 * C1).reshape(4, 128).T)
        m["b2d"] = np.ascontiguousarray((b2 * SE).reshape(2, 128).T)

        table = np.asarray(inputs["event_table"])
        allidx = np.concatenate([ctx.reshape(-1), ev.reshape(-1)])
        uniq, inv = np.unique(allidx, return_inverse=True)
        assert len(uniq) <= CT
        tuniq = np.asarray(table[uniq], np.float32) * SA
        ctab = np.zeros((CT, EP8), f8)
        ctab[: len(uniq), :] = tuniq[:, :EP8].astype(f8)
        rctx = inv[: ctx.size].reshape(BC, N, C)
        rev = inv[ctx.size :].reshape(BC, C)

        # remainder elems 256..299 of all 4 components, host-packed to the
        # moving layout [p, b*512 + i]: value = x_{r//64}[256 + r%64], r=2p+b
        remfull = np.zeros((BC, N, 256), np.float32)
        eremfull = np.zeros((128, 256), np.float32)
        for c in range(C):
            remfull[:, :, 64 * c : 64 * c + (E - 256)] = tuniq[rctx[:, :, c], EP8:E]
            eremfull[:BC, 64 * c : 64 * c + (E - 256)] = tuniq[rev[:, c], EP8:E]
        remd = (
            remfull.reshape(GROUPS, 512, 128, 2)
            .transpose(2, 0, 3, 1)
            .reshape(128, GROUPS * 1024)
        )
        m["remd"] = np.ascontiguousarray(remd).astype(f8)
        m["remed"] = np.ascontiguousarray(
            eremfull.reshape(128, 128, 2).transpose(1, 2, 0).reshape(128, 256)
        ).astype(f8)
        rctx = rctx.astype(np.int16)
        rev = rev.astype(np.int16)

        # context: per (g, c) gather of 512 idx, order i = z*128 + n
        ci = rctx.reshape(GROUPS, SUBT, N, C).transpose(0, 3, 1, 2)  # g,c,z,n
        cidx = np.concatenate(
            [
                _wrap16(ci[g, c].reshape(-1))
                for g in range(GROUPS)
                for c in range(C)
            ],
            axis=1,
        )
        # event: i = c*128 + b; b >= BC -> row 0 junk
        ei = np.zeros((C, 128), np.int16)
        ei[:, :BC] = rev.T
        m["ctab"] = ctab
        m["cidx"] = np.ascontiguousarray(cidx)
        m["eidx"] = np.ascontiguousarray(_wrap16(ei.reshape(-1)))
    else:
        bf = ml_dtypes.bfloat16
        w1t = np.zeros((CE, H1), np.float32)
        for c in range(C):
            w1t[EP * c : EP * c + E, :] = W1[:, E * c : E * (c + 1)].T
        wvt = np.zeros((CE, 9), np.float32)
        wvt[EP * 1 : EP * 1 + E, :] = Wv.T  # predicates = component 1
        m["w1t"] = w1t.astype(bf)
        m["wvt"] = wvt.astype(bf)
        m["w2t"] = np.ascontiguousarray(W2.T).astype(bf)
        m["b1d"] = np.ascontiguousarray(b1.reshape(4, 128).T)
        m["table"] = np.ascontiguousarray(
            np.asarray(inputs["event_table"], np.float32)
        )
        m["ctxidx"] = np.ascontiguousarray(
            ctx.astype(np.int32).transpose(1, 0, 2).reshape(128, BC * C)
        )
        m["evidx"] = ev.astype(np.int32)
    return m


def kernel(**inputs) -> np.ndarray:
    # fast path requires every shard's unique row count to fit int16
    fast = True
    ctx = np.asarray(inputs["batch_context"], np.int64)
    ev = np.asarray(inputs["batch_event"], np.int64)
    for core in range(NCORES):
        sl = slice(core * BC, (core + 1) * BC)
        nuniq = len(np.unique(np.concatenate(
            [ctx[sl].reshape(-1), ev[sl].reshape(-1)])))
        if nuniq > CT:
            fast = False
            break
    nc = _build_program(fast)
    in_maps = [_prep_core_inputs(inputs, core, fast) for core in range(NCORES)]
    res = run_bass_kernel_spmd(nc, in_maps, core_ids=list(range(NCORES)))
    return np.concatenate([r["out"] for r in res.results], axis=0)


if __name__ == "__main__":
    nc = _build_program(True)
    print("program built ok")
